# revision 1
# baseline (speedup 1.0000x reference)
"""Grouped-query attention (GQA) Trainium2 Bass kernel.

Problem: B=2, S=2048, DIM=2048, HQ=32, HKV=8, HEAD_DIM=64, causal mask.
Sharding: 8 cores = 2 (batch) x 4 (kv-head groups). Core c handles batch
c//4 and kv-block c%4 (2 kv heads, 8 q heads). Wq/Wk/Wv sharded
column-wise, Wo row-wise; each core writes a partial [S, DIM] output;
host sums the 4 partials per batch and adds bo.

On-chip dataflow (per core, all matmuls bf16 with fp32 PSUM accum):
  - q/k/v loaded TRANSPOSED from HBM via strided APs -> [d, s] tiles,
    cast to bf16 on DVE.
  - GEMM1: qxT[c,s] (Wq stationary), kxT[ck,s], vxT[ck,s]; v then
    PE-transposed to natural vx[j,hd] and packed with a ones column
    (flash-attention denominator trick).
  - GEMM2: scoresT[j,i] = kxT_h^T @ qxT_h, exp on ACT (no max
    subtraction -- scores are O(5) bounded), causal triangular mask
    applied multiplicatively post-exp on diagonal blocks only;
    j-blocks above the diagonal are skipped entirely.
  - GEMM3: attnT[c,i] (+denominator row) = vx1^T @ expT, accumulated
    over j-blocks in PSUM.
  - Normalize via reciprocal + SBUF broadcast-replicate DMA + DVE mul.
  - GEMM4: out[i,e] = attnT^T @ Wo_shard, written as fp32 partial.
"""

import numpy as np
import ml_dtypes

import concourse.bass as bass
import concourse.mybir as mybir
from concourse import bacc
from concourse.tile import TileContext
from concourse.bass_utils import run_bass_kernel_spmd

F32 = mybir.dt.float32
BF16 = mybir.dt.bfloat16
AF = mybir.ActivationFunctionType
ALU = mybir.AluOpType

B, S, DIM = 2, 2048, 2048
HQ, HKV, HD = 32, 8, 64
GROUP = HQ // HKV              # 4
NCORES = 8
KVSH = 4                       # kv-blocks (shards) per batch
CQ = (HQ // KVSH) * HD         # 512 q-proj cols per core (8 heads)
CK = (HKV // KVSH) * HD        # 128 kv-proj cols per core (2 heads)
NDC = DIM // 128               # 16 contraction chunks
NSS = S // 512                 # 4 sequence chunks of 512


def _t_ap(t, s0, d0, np_, nf):
    """AP reading DRAM [S, DIM] tensor transposed: partition=d (np_ rows
    at col d0), free=s (nf rows at row s0)."""
    base = t[0:1, 0:1]
    return bass.AP(tensor=base.tensor, offset=s0 * DIM + d0,
                   ap=[[1, np_], [DIM, nf]])


def _bcast_ap(ap, n):
    """Broadcast a [1, F] AP across n partitions (stride-0 partition)."""
    return bass.AP(tensor=ap.tensor, offset=ap.offset,
                   ap=[[0, n]] + list(ap.ap[1:]))


def build_nc(mode="causal"):
    nc = bacc.Bacc("TRN2", target_bir_lowering=False)

    q = nc.dram_tensor("q", [S, DIM], F32, kind="ExternalInput")
    k = nc.dram_tensor("k", [S, DIM], F32, kind="ExternalInput")
    v = nc.dram_tensor("v", [S, DIM], F32, kind="ExternalInput")
    wq = nc.dram_tensor("wq", [DIM, CQ], F32, kind="ExternalInput")
    wk = nc.dram_tensor("wk", [DIM, CK], F32, kind="ExternalInput")
    wv = nc.dram_tensor("wv", [DIM, CK], F32, kind="ExternalInput")
    wo = nc.dram_tensor("wo", [CQ, DIM], F32, kind="ExternalInput")
    bq = nc.dram_tensor("bq", [CQ], F32, kind="ExternalInput")
    bk = nc.dram_tensor("bk", [CK], F32, kind="ExternalInput")
    bv = nc.dram_tensor("bv", [CK], F32, kind="ExternalInput")
    tri = nc.dram_tensor("tri", [128, 128], BF16, kind="ExternalInput")
    ident = nc.dram_tensor("ident", [128, 128], BF16, kind="ExternalInput")
    mbias = None
    if mode == "dense":
        mbias = nc.dram_tensor("mbias", [S, S], F32, kind="ExternalInput")
    out = nc.dram_tensor("out", [S, DIM], F32, kind="ExternalOutput")

    with TileContext(nc) as tc:
        with (
            tc.tile_pool(name="consts", bufs=1) as consts,
            tc.tile_pool(name="w", bufs=1) as wpool,
            tc.tile_pool(name="wst", bufs=2) as wst,
            tc.tile_pool(name="stg", bufs=2) as stg,
            tc.tile_pool(name="xt", bufs=1) as xt,
            tc.tile_pool(name="acts", bufs=1) as acts,
            tc.tile_pool(name="exp", bufs=3) as expp,
            tc.tile_pool(name="nrm", bufs=2) as nrmp,
            tc.tile_pool(name="ob", bufs=2) as obp,
            tc.tile_pool(name="nat", bufs=1) as natp,
            tc.tile_pool(name="dr", bufs=2, space="DRAM") as drp,
            tc.tile_pool(name="ps2", bufs=2, space="PSUM") as ps2,
            tc.tile_pool(name="ps1", bufs=1, space="PSUM") as ps1,
        ):
            # ---- constants ----
            tri_t = consts.tile([128, 128], BF16, tag="tri")
            nc.gpsimd.dma_start(out=tri_t[:, :], in_=tri[:, :])
            id_t = consts.tile([128, 128], BF16, tag="id")
            nc.gpsimd.dma_start(out=id_t[:, :], in_=ident[:, :])
            bq_t = consts.tile([128, 4], F32, tag="bq")
            nc.gpsimd.dma_start(
                out=bq_t[:, :],
                in_=bass.AP(tensor=bq[0:1].tensor, offset=0,
                            ap=[[1, 128], [128, 4]]))
            bk_t = consts.tile([128, 1], F32, tag="bk")
            nc.gpsimd.dma_start(
                out=bk_t[:, :],
                in_=bass.AP(tensor=bk[0:1].tensor, offset=0,
                            ap=[[1, 128], [128, 1]]))
            bv_rep = consts.tile([128, 128], F32, tag="bv")
            nc.gpsimd.dma_start(
                out=bv_rep[:, :],
                in_=bass.AP(tensor=bv[0:1].tensor, offset=0,
                            ap=[[0, 128], [1, 128]]))

            # ---- weights: load fp32, cast to bf16 ----
            wq_bf, wk_bf, wv_bf, wo_bf = [], [], [], []
            for dc in range(NDC):
                st = stg.tile([128, 512], F32, tag="stg")
                nc.gpsimd.dma_start(out=st[:, :],
                                  in_=wq[dc * 128:(dc + 1) * 128, :])
                t = wpool.tile([128, CQ], BF16, tag=f"wq{dc}")
                nc.vector.tensor_copy(t[:, :], st[:, :])
                wq_bf.append(t)
            for dc in range(NDC):
                st = wst.tile([128, 256], F32, tag="wkv")
                nc.gpsimd.dma_start(out=st[:, 0:128],
                                  in_=wk[dc * 128:(dc + 1) * 128, :])
                nc.gpsimd.dma_start(out=st[:, 128:256],
                                  in_=wv[dc * 128:(dc + 1) * 128, :])
                tk = wpool.tile([128, CK], BF16, tag=f"wk{dc}")
                nc.vector.tensor_copy(tk[:, :], st[:, 0:128])
                wk_bf.append(tk)
                tv = wpool.tile([128, CK], BF16, tag=f"wv{dc}")
                nc.vector.tensor_copy(tv[:, :], st[:, 128:256])
                wv_bf.append(tv)
            for cc in range(4):
                t = wpool.tile([128, DIM], BF16, tag=f"wo{cc}")
                for hf in range(2):
                    st = wst.tile([128, 1024], F32, tag="wo")
                    nc.gpsimd.dma_start(
                        out=st[:, :],
                        in_=wo[cc * 128:(cc + 1) * 128,
                               hf * 1024:(hf + 1) * 1024])
                    nc.vector.tensor_copy(t[:, hf * 1024:(hf + 1) * 1024],
                                          st[:, :])
                wo_bf.append(t)

            # ---- persistent activations ----
            qxT = [acts.tile([128, S], BF16, tag=f"qx{cc}", name=f"qx{cc}") for cc in range(4)]
            kxT = acts.tile([128, S], BF16, tag="kx", name="kx")
            vxT = acts.tile([128, S], BF16, tag="vx", name="vx")
            attnT = [acts.tile([128, S], BF16, tag=f"at{cc}", name=f"at{cc}") for cc in range(4)]
            vx1 = [acts.tile([128, 130], BF16, tag=f"vp{sc}", name=f"vp{sc}")
                   for sc in range(S // 128)]

            for ss in range(NSS):
                s0 = ss * 512
                # ---- natural loads + bf16 casts + PE transpose ----
                qT, kT, vT = [], [], []
                for (src, lst, nm) in ((q, qT, "q"), (k, kT, "k"),
                                       (v, vT, "v")):
                    nats = []
                    for r in range(4):
                        st = stg.tile([128, 2048], F32, tag="stg")
                        nc.gpsimd.dma_start(
                            out=st[:, :],
                            in_=src[s0 + r * 128:s0 + (r + 1) * 128, :])
                        nb = natp.tile([128, 2048], BF16, tag=f"nb{r}",
                                       name=f"nb{r}")
                        nc.vector.tensor_copy(nb[:, :], st[:, :])
                        nats.append(nb)
                    for dc in range(NDC):
                        tp = ps2.tile([128, 512], BF16, tag="tp")
                        for r in range(4):
                            nc.tensor.transpose(
                                tp[:, r * 128:(r + 1) * 128],
                                nats[r][:, dc * 128:(dc + 1) * 128],
                                id_t[:, :])
                        t = xt.tile([128, 512], BF16, tag=f"{nm}T{dc}",
                                    name=f"{nm}T{dc}")
                        nc.vector.tensor_copy(t[:, :], tp[:, :])
                        lst.append(t)

                # ---- GEMM1: projections ----
                for cc in range(4):
                    ps = ps2.tile([128, 512], F32, tag="g1")
                    for dc in range(NDC):
                        nc.tensor.matmul(
                            ps[:, :], wq_bf[dc][:, cc * 128:(cc + 1) * 128],
                            qT[dc][:, :], start=(dc == 0), stop=(dc == NDC - 1))
                    nc.scalar.activation(qxT[cc][:, s0:s0 + 512], ps[:, :],
                                         AF.Identity, bias=bq_t[:, cc:cc + 1])
                ps = ps2.tile([128, 512], F32, tag="g1")
                for dc in range(NDC):
                    nc.tensor.matmul(ps[:, :], wk_bf[dc][:, :], kT[dc][:, :],
                                     start=(dc == 0), stop=(dc == NDC - 1))
                nc.scalar.activation(kxT[:, s0:s0 + 512], ps[:, :],
                                     AF.Identity, bias=bk_t[:, 0:1])
                ps = ps2.tile([128, 512], F32, tag="g1")
                for dc in range(NDC):
                    nc.tensor.matmul(ps[:, :], wv_bf[dc][:, :], vT[dc][:, :],
                                     start=(dc == 0), stop=(dc == NDC - 1))
                nc.scalar.activation(vxT[:, s0:s0 + 512], ps[:, :], AF.Copy)

                # ---- v: PE transpose to natural + ones column ----
                vtp = ps2.tile([128, 512], BF16, tag="tp")
                for sc in range(4):
                    nc.tensor.transpose(
                        vtp[:, sc * 128:(sc + 1) * 128],
                        vxT[:, s0 + sc * 128:s0 + (sc + 1) * 128],
                        id_t[:, :])
                for sc in range(4):
                    jb = ss * 4 + sc
                    vx = vx1[jb]
                    for h2 in range(2):
                        nc.vector.tensor_tensor(
                            vx[:, h2 * 65:h2 * 65 + 64],
                            vtp[:, sc * 128 + h2 * 64:sc * 128 + (h2 + 1) * 64],
                            bv_rep[:, h2 * 64:(h2 + 1) * 64], ALU.add)
                    nc.vector.memset(vx[:, 64:65], 1.0)
                    nc.vector.memset(vx[:, 129:130], 1.0)

                # ---- attention for i-block [s0, s0+512) ----
                njb = 4 * (ss + 1) if mode == "causal" else S // 128
                for h in range(8):
                    # head h lives in tile h%4 at partition (h//4)*64, so its
                    # partition base always equals its kv head's base in kxT
                    # (matmul requires equal base partitions). Host permutes
                    # Wq columns / Wo rows to match this layout.
                    th, po, kv = h % 4, (h // GROUP) * 64, h // GROUP
                    at = ps1.tile([65, 512], F32, tag="at")
                    for jb in range(njb):
                        j0 = jb * 128
                        off = max(0, j0 - s0) if mode == "causal" else 0
                        N = 512 - off
                        sp = ps2.tile([128, 512], F32, tag="sc")
                        nc.tensor.matmul(
                            sp[:, :N],
                            kxT[kv * 64:(kv + 1) * 64, j0:j0 + 128],
                            qxT[th][po:po + 64, s0 + off:s0 + 512],
                            start=True, stop=True)
                        if mode == "dense":
                            mb = nrmp.tile([128, 512], F32, tag="mb")
                            nc.gpsimd.dma_start(
                                out=mb[:, :N],
                                in_=mbias[j0:j0 + 128, s0 + off:s0 + 512])
                            nc.vector.tensor_tensor(sp[:, :N], sp[:, :N],
                                                    mb[:, :N], ALU.add)
                        ex = expp.tile([128, 512], BF16, tag="exp")
                        nc.scalar.activation(ex[:, :N], sp[:, :N], AF.Exp,
                                             scale=0.125)
                        if mode == "causal" and j0 >= s0:
                            nc.vector.tensor_tensor(ex[:, 0:128], ex[:, 0:128],
                                                    tri_t[:, :], ALU.mult)
                        nc.tensor.matmul(
                            at[:, off:512], vx1[jb][:, kv * 65:kv * 65 + 65],
                            ex[:, :N], start=(jb == 0), stop=(jb == njb - 1))
                    # normalize by denominator row (64) and store bf16
                    nm = nrmp.tile([65, 512], F32, tag="nrm")
                    nc.vector.reciprocal(nm[64:65, :], at[64:65, :])
                    dr = drp.tile([1, 512], F32, tag="dn")
                    nc.gpsimd.dma_start(out=dr[0:1, :], in_=nm[64:65, :])
                    nc.gpsimd.dma_start(out=nm[0:64, :],
                                        in_=_bcast_ap(dr[0:1, :], 64))
                    nc.vector.tensor_tensor(
                        attnT[th][po:po + 64, s0:s0 + 512],
                        at[0:64, :], nm[0:64, :], ALU.mult)

                # ---- GEMM4: output projection (partial) ----
                for sc in range(4):
                    i0 = s0 + sc * 128
                    for hf in range(2):
                        ob = obp.tile([128, 1024], F32, tag="ob")
                        for e2 in range(2):
                            ec = hf * 2 + e2
                            g4 = ps1.tile([128, 512], F32, tag="g4")
                            for cc2 in range(4):
                                nc.tensor.matmul(
                                    g4[:, :], attnT[cc2][:, i0:i0 + 128],
                                    wo_bf[cc2][:, ec * 512:(ec + 1) * 512],
                                    start=(cc2 == 0), stop=(cc2 == 3))
                            nc.scalar.activation(
                                ob[:, e2 * 512:(e2 + 1) * 512], g4[:, :],
                                AF.Copy)
                        nc.gpsimd.dma_start(
                            out=out[i0:i0 + 128, hf * 1024:(hf + 1) * 1024],
                            in_=ob[:, :])
    nc.finalize()
    return nc


_CACHE = {}


def _get_nc(mode):
    if mode not in _CACHE:
        _CACHE[mode] = build_nc(mode)
    return _CACHE[mode]


def kernel(q, k, v, mask, Wq, bq, Wk, bk, Wv, bv, Wo, bo):
    q = np.asarray(q, np.float32)
    k = np.asarray(k, np.float32)
    v = np.asarray(v, np.float32)
    mask = np.asarray(mask)
    Wq = np.asarray(Wq, np.float32)
    Wk = np.asarray(Wk, np.float32)
    Wv = np.asarray(Wv, np.float32)
    Wo = np.asarray(Wo, np.float32)
    bq = np.asarray(bq, np.float32)
    bk = np.asarray(bk, np.float32)
    bv = np.asarray(bv, np.float32)
    bo = np.asarray(bo, np.float32)

    m = mask.astype(np.float64)
    if np.array_equal(m, np.tril(np.ones((S, S)))):
        mode = "causal"
    elif np.all(m == 1):
        mode = "none"
    else:
        mode = "dense"

    nc = _get_nc(mode)
    tri_np = np.triu(np.ones((128, 128))).astype(ml_dtypes.bfloat16)
    id_np = np.eye(128).astype(ml_dtypes.bfloat16)

    # On-chip layout places local q head h in tile h%4 at partition
    # (h//4)*64 so q/k partition bases match in the scores matmul. Permute
    # Wq columns / Wo rows / bq accordingly: tile cc holds heads (cc, cc+4).
    head_perm = [h for cc in range(4) for h in (cc, cc + 4)]
    col_perm = np.concatenate(
        [np.arange(h * HD, (h + 1) * HD) for h in head_perm])

    in_maps = []
    for core in range(NCORES):
        b, kb = core // KVSH, core % KVSH
        wq_sh = Wq[:, kb * CQ:(kb + 1) * CQ][:, col_perm]
        wo_sh = Wo[kb * CQ:(kb + 1) * CQ, :][col_perm, :]
        bq_sh = bq[kb * CQ:(kb + 1) * CQ][col_perm]
        im = {
            "q": np.ascontiguousarray(q[b]),
            "k": np.ascontiguousarray(k[b]),
            "v": np.ascontiguousarray(v[b]),
            "wq": np.ascontiguousarray(wq_sh),
            "wk": np.ascontiguousarray(Wk[:, kb * CK:(kb + 1) * CK]),
            "wv": np.ascontiguousarray(Wv[:, kb * CK:(kb + 1) * CK]),
            "wo": np.ascontiguousarray(wo_sh),
            "bq": np.ascontiguousarray(bq_sh),
            "bk": np.ascontiguousarray(bk[kb * CK:(kb + 1) * CK]),
            "bv": np.ascontiguousarray(bv[kb * CK:(kb + 1) * CK]),
            "tri": tri_np,
            "ident": id_np,
        }
        if mode == "dense":
            with np.errstate(divide="ignore"):
                bias = -(1.0 / mask.astype(np.float32) + 1.0)
            im["mbias"] = np.ascontiguousarray(bias.T * 8.0)
        in_maps.append(im)

    res = run_bass_kernel_spmd(nc, in_maps, core_ids=list(range(NCORES)))
    outs = [r["out"] for r in res.results]
    full = np.empty((B, S, DIM), np.float32)
    for b in range(B):
        acc = outs[b * KVSH].astype(np.float32)
        for kb in range(1, KVSH):
            acc = acc + outs[b * KVSH + kb]
        full[b] = acc + bo[None, :]
    return full



# revision 13
# speedup vs baseline: 2.1232x; 2.1232x over previous
"""Grouped-query attention (GQA) Trainium2 Bass kernel, v2.

Problem: B=2, S=2048, DIM=2048, HQ=32, HKV=8, HEAD_DIM=64, causal mask.
Sharding: 8 cores = 2 (batch) x 4 (kv-head groups). Core c handles batch
c//4 and kv-block c%4 (2 kv heads, 8 q heads). Wq/Wk/Wv sharded
column-wise, Wo row-wise; each core writes a partial [S, DIM] fp16
output; host sums the 4 partials per batch and adds bo.

v2 changes vs v1:
  - q/k/v pre-transposed and pre-cast to bf16 on host -> no on-chip
    casts/PE transposes; weights pre-cast to bf16 (halves DMA bytes).
  - Bulk 3D-AP DMAs (one per tensor per seq chunk) instead of 48 small
    ones.
  - Score matmuls for the two kv-head groups issued back-to-back at
    partition bases 0/64 -> concurrent PE row-tiles (2x on K=64 work).
  - exp over [128,1024] (both heads of a pair) per j-block.
  - Denominator: reciprocal_approx_fast + PE outer-product broadcast
    (no DRAM round-trip, no slow DVE reciprocal).
  - GEMM1/GEMM4 evacuations on DVE (tensor_scalar add for bias), exp
    exclusively on ACT.
  - GEMM4(ss-1) and GEMM1(ss+1) chains interleaved into the attention
    pair loop to fill PE while ACT computes exp.
  - fp16 partial outputs (halves output DMA).
"""

import numpy as np
import ml_dtypes

import concourse.bass as bass
import concourse.mybir as mybir
from concourse import bacc
from concourse.tile import TileContext
from concourse.bass_utils import run_bass_kernel_spmd

F32 = mybir.dt.float32
F16 = mybir.dt.float16
BF16 = mybir.dt.bfloat16
AF = mybir.ActivationFunctionType
ALU = mybir.AluOpType

B, S, DIM = 2, 2048, 2048
HQ, HKV, HD = 32, 8, 64
GROUP = HQ // HKV              # 4
NCORES = 8
KVSH = 4                       # kv-blocks (shards) per batch
CQ = (HQ // KVSH) * HD         # 512 q-proj cols per core (8 heads)
CK = (HKV // KVSH) * HD        # 128 kv-proj cols per core (2 heads)
NDC = DIM // 128               # 16 contraction chunks
NSS = S // 512                 # 4 sequence chunks of 512


def build_nc(mode="causal"):
    nc = bacc.Bacc("TRN2", target_bir_lowering=False)

    qT = nc.dram_tensor("qT", [DIM, S], BF16, kind="ExternalInput")
    kT = nc.dram_tensor("kT", [DIM, S], BF16, kind="ExternalInput")
    vT = nc.dram_tensor("vT", [DIM, S], BF16, kind="ExternalInput")
    wq = nc.dram_tensor("wq", [DIM, CQ], BF16, kind="ExternalInput")
    wkv = nc.dram_tensor("wkv", [DIM, 2 * CK], BF16, kind="ExternalInput")
    wo = nc.dram_tensor("wo", [CQ, DIM], BF16, kind="ExternalInput")
    bq = nc.dram_tensor("bq", [CQ], F32, kind="ExternalInput")
    bkv = nc.dram_tensor("bkv", [2 * CK], F32, kind="ExternalInput")
    tri = nc.dram_tensor("tri", [128, 128], BF16, kind="ExternalInput")
    ident = nc.dram_tensor("ident", [128, 128], BF16, kind="ExternalInput")
    mbias = None
    if mode == "dense":
        mbias = nc.dram_tensor("mbias", [S, S], F32, kind="ExternalInput")
    out = nc.dram_tensor("out", [S, DIM], F16, kind="ExternalOutput")

    def dram3(t, d0, nd, s0, ns):
        """3D AP over a [DIM, S] dram tensor: nd chunks of 128 rows
        starting at row d0, ns cols from s0. Lands in an SBUF tile
        [128, nd*ns] chunk-major."""
        return bass.AP(tensor=t[0:1, 0:1].tensor, offset=d0 * S + s0,
                       ap=[[S, 128], [128 * S, nd], [1, ns]])

    with TileContext(nc) as tc:
        with (
            tc.tile_pool(name="consts", bufs=1) as consts,
            tc.tile_pool(name="w", bufs=1) as wpool,
            tc.tile_pool(name="xs", bufs=1) as xs,
            tc.tile_pool(name="acts", bufs=1) as acts,
            tc.tile_pool(name="qx", bufs=2) as qxp,
            tc.tile_pool(name="kx", bufs=2) as kxp,
            tc.tile_pool(name="vx", bufs=2) as vxp,
            tc.tile_pool(name="an", bufs=3) as anp,
            tc.tile_pool(name="exp", bufs=3) as expp,
            tc.tile_pool(name="nm", bufs=2) as nmp,
            tc.tile_pool(name="ob", bufs=2) as obp,
            tc.tile_pool(name="mbp", bufs=2) as mbp,
            tc.tile_pool(name="sc", bufs=2, space="PSUM") as scp,
            tc.tile_pool(name="at", bufs=1, space="PSUM") as atp,
            tc.tile_pool(name="acc", bufs=2, space="PSUM") as accp,
        ):
            # ---- constants ----
            tri_t = consts.tile([128, 128], BF16, tag="tri")
            nc.gpsimd.dma_start(out=tri_t[:, :], in_=tri[:, :])
            id_t = consts.tile([128, 128], BF16, tag="id")
            nc.gpsimd.dma_start(out=id_t[:, :], in_=ident[:, :])
            bq_t = consts.tile([128, 4], F32, tag="bq")
            nc.gpsimd.dma_start(
                out=bq_t[:, :],
                in_=bass.AP(tensor=bq[0:1].tensor, offset=0,
                            ap=[[1, 128], [128, 4]]))
            bkv_t = consts.tile([128, 2], F32, tag="bkv")
            nc.gpsimd.dma_start(
                out=bkv_t[:, :],
                in_=bass.AP(tensor=bkv[0:1].tensor, offset=0,
                            ap=[[1, 128], [128, 2]]))

            # ---- weights: single bulk DMA each, already bf16 ----
            # (wq/wkv issued here; wo deferred until after the first input
            # loads so GEMM1(0) isn't stuck behind 2MB it doesn't need)
            wq_t = wpool.tile([128, NDC * CQ], BF16, tag="wq")
            wkv_t = wpool.tile([128, NDC * 2 * CK], BF16, tag="wkv")
            wo_t = wpool.tile([128, 4 * DIM], BF16, tag="wo")

            def load_wq_wkv():
                nc.gpsimd.dma_start(
                    out=wq_t[:, :],
                    in_=bass.AP(tensor=wq[0:1, 0:1].tensor, offset=0,
                                ap=[[CQ, 128], [128 * CQ, NDC], [1, CQ]]))
                nc.gpsimd.dma_start(
                    out=wkv_t[:, :],
                    in_=bass.AP(tensor=wkv[0:1, 0:1].tensor, offset=0,
                                ap=[[2 * CK, 128], [128 * 2 * CK, NDC],
                                    [1, 2 * CK]]))

            def load_wo():
                nc.gpsimd.dma_start(
                    out=wo_t[:, :],
                    in_=bass.AP(tensor=wo[0:1, 0:1].tensor, offset=0,
                                ap=[[DIM, 128], [128 * DIM, 4], [1, DIM]]))

            def wq_ap(dc, cc):
                return wq_t[:, dc * CQ + cc * 128:dc * CQ + (cc + 1) * 128]

            def wk_ap(dc):
                return wkv_t[:, dc * 2 * CK:dc * 2 * CK + CK]

            def wv_ap(dc):
                return wkv_t[:, dc * 2 * CK + CK:(dc + 1) * 2 * CK]

            def wo_ap(cc, ec):
                return wo_t[:, cc * DIM + ec * 512:cc * DIM + (ec + 1) * 512]

            # ---- persistent activations ----
            # layout: qxT_ss[ss][cc] [128, 512]: partitions 0:64 = head cc
            # (kv group 0), 64:128 = head cc+4 (kv group 1). Host permutes
            # Wq cols / Wo rows to match.
            vx1 = [acts.tile([128, 130], BF16, tag=f"vp{jb}", name=f"vp{jb}")
                   for jb in range(S // 128)]
            qxT = [[None] * 4 for _ in range(NSS)]
            kxT = [None] * NSS
            attnT = [[None] * 4 for _ in range(NSS)]

            # input stream tiles (one ss in flight per tensor)
            def load_inputs(ss):
                # 3 bulk DMAs; single-buffered tiles: by the time load(ss)
                # is issued, GEMM1(ss-1) has consumed the previous content.
                s0 = ss * 512
                xq = xs.tile([128, NDC * 512], BF16, tag="xq", bufs=1,
                             name=f"xq{ss}")
                nc.gpsimd.dma_start(out=xq[:, :],
                                    in_=dram3(qT, 0, NDC, s0, 512))
                xk = xs.tile([128, NDC * 512], BF16, tag="xk", bufs=1,
                             name=f"xk{ss}")
                nc.gpsimd.dma_start(out=xk[:, :],
                                    in_=dram3(kT, 0, NDC, s0, 512))
                xv = xs.tile([128, NDC * 512], BF16, tag="xv", bufs=1,
                             name=f"xv{ss}")
                nc.gpsimd.dma_start(out=xv[:, :],
                                    in_=dram3(vT, 0, NDC, s0, 512))
                return xq, xk, xv

            def gemm1_closures(ss, xq, xk, xv):
                """Returns a list of closures, each issuing one projection
                chain for chunk ss."""
                s0 = ss * 512
                cls = []

                def q_chain(cc):
                    def run():
                        ps = accp.tile([128, 512], F32, tag="acc",
                                       name=f"g1q{ss}{cc}")
                        for dc in range(NDC):
                            nc.tensor.matmul(
                                ps[:, :], wq_ap(dc, cc),
                                xq[:, dc * 512:(dc + 1) * 512],
                                start=(dc == 0), stop=(dc == NDC - 1))
                        if mode == "causal":
                            t = qxp.tile([128, 512], BF16, tag=f"qx{cc}",
                                         name=f"qx{ss}{cc}")
                        else:
                            # flat schedule: all chunks' qxT live at once
                            t = qxp.tile([128, 512], BF16,
                                         tag=f"qx{ss}{cc}", bufs=1,
                                         name=f"qx{ss}{cc}")
                        nc.vector.tensor_scalar_add(t[:, :], ps[:, :],
                                                    bq_t[:, cc:cc + 1])
                        qxT[ss][cc] = t
                    return run

                for cc in range(4):
                    cls.append(q_chain(cc))

                def k_chain():
                    ps = accp.tile([128, 512], F32, tag="acc",
                                   name=f"g1k{ss}")
                    for dc in range(NDC):
                        nc.tensor.matmul(ps[:, :], wk_ap(dc),
                                         xk[:, dc * 512:(dc + 1) * 512],
                                         start=(dc == 0),
                                         stop=(dc == NDC - 1))
                    # kxT persists across the whole kernel (attention at
                    # chunk ss reads all chunks <= ss) -> per-ss tag.
                    t = kxp.tile([128, 512], BF16, tag=f"kx{ss}", bufs=1,
                                 name=f"kx{ss}")
                    nc.vector.tensor_scalar_add(t[:, :], ps[:, :],
                                                bkv_t[:, 0:1])
                    kxT[ss] = t
                cls.append(k_chain)

                def v_chain():
                    ps = accp.tile([128, 512], F32, tag="acc",
                                   name=f"g1v{ss}")
                    for dc in range(NDC):
                        nc.tensor.matmul(ps[:, :], wv_ap(dc),
                                         xv[:, dc * 512:(dc + 1) * 512],
                                         start=(dc == 0),
                                         stop=(dc == NDC - 1))
                    vxs = vxp.tile([128, 512], BF16, tag="vxT",
                                   name=f"vxT{ss}")
                    nc.vector.tensor_scalar_add(vxs[:, :], ps[:, :],
                                                bkv_t[:, 1:2])
                    # transpose to natural [j, c] and pack into vx1 with
                    # ones columns (flash denominator trick)
                    vtp = accp.tile([128, 512], BF16, tag="acc",
                                    name=f"vtp{ss}")
                    for sc in range(4):
                        nc.tensor.transpose(
                            vtp[:, sc * 128:(sc + 1) * 128],
                            vxs[:, sc * 128:(sc + 1) * 128], id_t[:, :])
                    for sc in range(4):
                        vx = vx1[ss * 4 + sc]
                        nc.vector.tensor_copy(
                            vx[:, 0:64], vtp[:, sc * 128:sc * 128 + 64])
                        nc.vector.tensor_copy(
                            vx[:, 65:129],
                            vtp[:, sc * 128 + 64:(sc + 1) * 128])
                        nc.vector.memset(vx[:, 64:65], 1.0)
                        nc.vector.memset(vx[:, 129:130], 1.0)
                cls.append(v_chain)
                return cls

            def gemm4_closures(ss):
                """Output projection for chunk ss (reads attnT[ss])."""
                s0 = ss * 512
                cls = []

                def one(sc):
                    def run():
                        i0 = s0 + sc * 128
                        ob = obp.tile([128, DIM], F16, tag="ob",
                                      name=f"ob{ss}{sc}")
                        for ec in range(4):
                            g4 = accp.tile([128, 512], F32, tag="acc",
                                           name=f"g4{ss}{sc}{ec}")
                            for cc2 in range(4):
                                nc.tensor.matmul(
                                    g4[:, :],
                                    attnT[ss][cc2][:, sc * 128:(sc + 1) * 128],
                                    wo_ap(cc2, ec),
                                    start=(cc2 == 0), stop=(cc2 == 3))
                            nc.vector.tensor_copy(
                                ob[:, ec * 512:(ec + 1) * 512], g4[:, :])
                        nc.sync.dma_start(out=out[i0:i0 + 128, :],
                                          in_=ob[:, :])
                    return run

                for sc in range(4):
                    cls.append(one(sc))
                return cls

            def pair_loop(ss, bg):
                """Attention for i-chunk ss, all 4 head pairs; injects
                background closures from bg between j-steps."""
                s0 = ss * 512
                njb = 4 * (ss + 1) if mode == "causal" else S // 128
                for cc in range(4):
                    atl = atp.tile([65, 512], F32, tag="atl",
                                   name=f"atl{ss}{cc}")
                    ath = atp.tile([65, 512], F32, tag="ath",
                                   name=f"ath{ss}{cc}")
                    for jb in range(njb):
                        j0 = jb * 128
                        off = max(0, j0 - s0) if mode == "causal" else 0
                        N = 512 - off
                        kx = kxT[jb // 4]
                        kl = kx[0:64, (jb % 4) * 128:(jb % 4 + 1) * 128]
                        kh = kx[64:128, (jb % 4) * 128:(jb % 4 + 1) * 128]
                        sct = scp.tile([128, 1024], F32, tag="sc",
                                       name=f"sc{ss}{cc}{jb}")
                        nc.tensor.matmul(
                            sct[:, 0:N], kl,
                            qxT[ss][cc][0:64, off:512],
                            start=True, stop=True)
                        nc.tensor.matmul(
                            sct[:, 512:512 + N], kh,
                            qxT[ss][cc][64:128, off:512],
                            start=True, stop=True)
                        if mode == "dense":
                            mb = mbp.tile([128, 512], F32, tag="mb",
                                          name=f"mb{ss}{cc}{jb}")
                            nc.gpsimd.dma_start(
                                out=mb[:, :N],
                                in_=mbias[j0:j0 + 128, s0 + off:s0 + 512])
                            nc.vector.tensor_tensor(
                                sct[:, 0:N], sct[:, 0:N], mb[:, :N],
                                ALU.add)
                            nc.vector.tensor_tensor(
                                sct[:, 512:512 + N], sct[:, 512:512 + N],
                                mb[:, :N], ALU.add)
                        ex = expp.tile([128, 1024], BF16, tag="ex",
                                       name=f"ex{ss}{cc}{jb}")
                        if off == 0:
                            nc.scalar.activation(ex[:, :], sct[:, :],
                                                 AF.Exp, scale=0.125)
                        else:
                            nc.scalar.activation(ex[:, 0:N], sct[:, 0:N],
                                                 AF.Exp, scale=0.125)
                            nc.scalar.activation(ex[:, 512:512 + N],
                                                 sct[:, 512:512 + N],
                                                 AF.Exp, scale=0.125)
                        if mode == "causal" and j0 >= s0:
                            nc.vector.tensor_tensor(
                                ex[:, 0:128], ex[:, 0:128], tri_t[:, :],
                                ALU.mult)
                            nc.vector.tensor_tensor(
                                ex[:, 512:640], ex[:, 512:640], tri_t[:, :],
                                ALU.mult)
                        nc.tensor.matmul(
                            atl[:, off:512], vx1[jb][:, 0:65], ex[:, 0:N],
                            start=(jb == 0), stop=(jb == njb - 1))
                        nc.tensor.matmul(
                            ath[:, off:512], vx1[jb][:, 65:130],
                            ex[:, 512:512 + N],
                            start=(jb == 0), stop=(jb == njb - 1))
                        if bg and jb % 2 == 1:
                            bg.pop(0)()
                    # ---- normalize both heads of the pair ----
                    # reciprocal of denominator row, partition-broadcast on
                    # GPSIMD (SBUF->SBUF, keeps PE out of the path), then
                    # one DVE mult per head (PSUM x SBUF -> SBUF).
                    att = anp.tile([128, 512], BF16, tag=f"an{cc}",
                                   name=f"an{ss}{cc}")
                    for hi, at in ((0, atl), (1, ath)):
                        # approx-recip mis-reads PSUM at partition 64;
                        # stage the denominator row through SBUF first.
                        den = nmp.tile([1, 512], F32, tag="den",
                                       name=f"den{ss}{cc}{hi}")
                        nc.vector.tensor_copy(den[0:1, :], at[64:65, :])
                        nmf = nmp.tile([1, 512], F32, tag="nmf",
                                       name=f"nmf{ss}{cc}{hi}")
                        nc.vector.reciprocal_approx_fast(nmf[0:1, :],
                                                         den[0:1, :])
                        nmb = nmp.tile([64, 512], F32, tag="nmb",
                                       name=f"nmb{ss}{cc}{hi}")
                        nc.gpsimd.partition_broadcast(nmb[:, :],
                                                      nmf[0:1, :])
                        nc.vector.tensor_tensor(
                            att[hi * 64:(hi + 1) * 64, :], at[0:64, :],
                            nmb[:, :], ALU.mult)
                    attnT[ss][cc] = att
                    if bg:
                        bg.pop(0)()

            # ================= main schedule =================
            if mode == "causal":
                # software-pipelined: GEMM1(ss+1) + GEMM4(ss-1) chains are
                # injected into chunk ss's attention loop to fill PE while
                # ACT computes exp.
                xin = load_inputs(0)
                for c in gemm1_closures(0, *xin):
                    c()
                for ss in range(NSS):
                    bg = []
                    if ss > 0:
                        bg.extend(gemm4_closures(ss - 1))
                    if ss + 1 < NSS:
                        xin = load_inputs(ss + 1)
                        bg.extend(gemm1_closures(ss + 1, *xin))
                    pair_loop(ss, bg)
                    for c in bg:
                        c()
                for c in gemm4_closures(NSS - 1):
                    c()
            else:
                # non-causal: attention at any chunk reads ALL k/v chunks,
                # so all projections must complete first (flat schedule).
                for ss in range(NSS):
                    xin = load_inputs(ss)
                    for c in gemm1_closures(ss, *xin):
                        c()
                for ss in range(NSS):
                    bg = gemm4_closures(ss - 1) if ss > 0 else []
                    pair_loop(ss, bg)
                    for c in bg:
                        c()
                for c in gemm4_closures(NSS - 1):
                    c()

    nc.finalize()
    return nc


_CACHE = {}


def _get_nc(mode):
    if mode not in _CACHE:
        _CACHE[mode] = build_nc(mode)
    return _CACHE[mode]


def _to_bf16(x):
    """Fast float32 -> bfloat16 with round-to-nearest-even."""
    x = np.ascontiguousarray(x, np.float32)
    u = x.view(np.uint32)
    r = ((u + 0x7FFF + ((u >> 16) & 1)) >> 16).astype(np.uint16)
    return r.view(ml_dtypes.bfloat16)


def kernel(q, k, v, mask, Wq, bq, Wk, bk, Wv, bv, Wo, bo):
    q = np.asarray(q, np.float32)
    k = np.asarray(k, np.float32)
    v = np.asarray(v, np.float32)
    mask = np.asarray(mask)
    Wq = np.asarray(Wq, np.float32)
    Wk = np.asarray(Wk, np.float32)
    Wv = np.asarray(Wv, np.float32)
    Wo = np.asarray(Wo, np.float32)
    bq = np.asarray(bq, np.float32)
    bk = np.asarray(bk, np.float32)
    bv = np.asarray(bv, np.float32)
    bo = np.asarray(bo, np.float32)

    m = mask.astype(np.float64)
    if np.array_equal(m, np.tril(np.ones((S, S)))):
        mode = "causal"
    elif np.all(m == 1):
        mode = "none"
    else:
        mode = "dense"

    nc = _get_nc(mode)
    tri_np = np.triu(np.ones((128, 128))).astype(ml_dtypes.bfloat16)
    id_np = np.eye(128).astype(ml_dtypes.bfloat16)

    # transposed bf16 copies of q/k/v, shared across the 4 cores per batch
    qT = [_to_bf16(q[b].T) for b in range(B)]
    kT = [_to_bf16(k[b].T) for b in range(B)]
    vT = [_to_bf16(v[b].T) for b in range(B)]

    # On-chip layout places local q head h in tile h%4 at partition
    # (h//4)*64 so q/k partition bases match in the scores matmul. Permute
    # Wq columns / Wo rows / bq accordingly: tile cc holds heads (cc, cc+4).
    head_perm = [h for cc in range(4) for h in (cc, cc + 4)]
    col_perm = np.concatenate(
        [np.arange(h * HD, (h + 1) * HD) for h in head_perm])

    in_maps = []
    for core in range(NCORES):
        b, kb = core // KVSH, core % KVSH
        wq_sh = Wq[:, kb * CQ:(kb + 1) * CQ][:, col_perm]
        wo_sh = Wo[kb * CQ:(kb + 1) * CQ, :][col_perm, :]
        bq_sh = bq[kb * CQ:(kb + 1) * CQ][col_perm]
        wkv_sh = np.concatenate(
            [Wk[:, kb * CK:(kb + 1) * CK], Wv[:, kb * CK:(kb + 1) * CK]],
            axis=1)
        bkv_sh = np.concatenate(
            [bk[kb * CK:(kb + 1) * CK], bv[kb * CK:(kb + 1) * CK]])
        im = {
            "qT": qT[b],
            "kT": kT[b],
            "vT": vT[b],
            "wq": _to_bf16(wq_sh),
            "wkv": _to_bf16(wkv_sh),
            "wo": _to_bf16(wo_sh),
            "bq": np.ascontiguousarray(bq_sh),
            "bkv": bkv_sh,
            "tri": tri_np,
            "ident": id_np,
        }
        if mode == "dense":
            with np.errstate(divide="ignore"):
                bias = -(1.0 / mask.astype(np.float32) + 1.0)
            im["mbias"] = np.ascontiguousarray(bias.T * 8.0)
        in_maps.append(im)

    res = run_bass_kernel_spmd(nc, in_maps, core_ids=list(range(NCORES)))
    outs = [r["out"] for r in res.results]
    full = np.empty((B, S, DIM), np.float32)
    for b in range(B):
        acc = outs[b * KVSH].astype(np.float32)
        for kb in range(1, KVSH):
            acc = acc + outs[b * KVSH + kb]
        full[b] = acc + bo[None, :]
    return full


# revision 17
# speedup vs baseline: 2.3924x; 1.1268x over previous
"""Grouped-query attention (GQA) Trainium2 Bass kernel, v2.

Problem: B=2, S=2048, DIM=2048, HQ=32, HKV=8, HEAD_DIM=64, causal mask.
Sharding: 8 cores = 2 (batch) x 4 (kv-head groups). Core c handles batch
c//4 and kv-block c%4 (2 kv heads, 8 q heads). Wq/Wk/Wv sharded
column-wise, Wo row-wise; each core writes a partial [S, DIM] fp16
output; host sums the 4 partials per batch and adds bo.

v2 changes vs v1:
  - q/k/v pre-transposed and pre-cast to bf16 on host -> no on-chip
    casts/PE transposes; weights pre-cast to bf16 (halves DMA bytes).
  - Bulk 3D-AP DMAs (one per tensor per seq chunk) instead of 48 small
    ones.
  - Score matmuls for the two kv-head groups issued back-to-back at
    partition bases 0/64 -> concurrent PE row-tiles (2x on K=64 work).
  - exp over [128,1024] (both heads of a pair) per j-block.
  - Denominator: reciprocal_approx_fast + PE outer-product broadcast
    (no DRAM round-trip, no slow DVE reciprocal).
  - GEMM1/GEMM4 evacuations on DVE (tensor_scalar add for bias), exp
    exclusively on ACT.
  - GEMM4(ss-1) and GEMM1(ss+1) chains interleaved into the attention
    pair loop to fill PE while ACT computes exp.
  - fp16 partial outputs (halves output DMA).
"""

import numpy as np
import ml_dtypes

import concourse.bass as bass
import concourse.mybir as mybir
from concourse import bacc
from concourse.tile import TileContext
from concourse.bass_utils import run_bass_kernel_spmd

F32 = mybir.dt.float32
F16 = mybir.dt.float16
BF16 = mybir.dt.bfloat16
AF = mybir.ActivationFunctionType
ALU = mybir.AluOpType

B, S, DIM = 2, 2048, 2048
HQ, HKV, HD = 32, 8, 64
GROUP = HQ // HKV              # 4
NCORES = 8
KVSH = 4                       # kv-blocks (shards) per batch
CQ = (HQ // KVSH) * HD         # 512 q-proj cols per core (8 heads)
CK = (HKV // KVSH) * HD        # 128 kv-proj cols per core (2 heads)
NDC = DIM // 128               # 16 contraction chunks
NSS = S // 512                 # 4 sequence chunks of 512


def build_nc(mode="causal"):
    nc = bacc.Bacc("TRN2", target_bir_lowering=False)

    qT = nc.dram_tensor("qT", [DIM, S], BF16, kind="ExternalInput")
    kT = nc.dram_tensor("kT", [DIM, S], BF16, kind="ExternalInput")
    vT = nc.dram_tensor("vT", [DIM, S], BF16, kind="ExternalInput")
    wq = nc.dram_tensor("wq", [DIM, CQ], BF16, kind="ExternalInput")
    wkv = nc.dram_tensor("wkv", [DIM, 2 * CK], BF16, kind="ExternalInput")
    wo = nc.dram_tensor("wo", [CQ, DIM], BF16, kind="ExternalInput")
    bq = nc.dram_tensor("bq", [CQ], F32, kind="ExternalInput")
    bkv = nc.dram_tensor("bkv", [2 * CK], F32, kind="ExternalInput")
    tri = nc.dram_tensor("tri", [128, 128], BF16, kind="ExternalInput")
    ident = nc.dram_tensor("ident", [128, 128], BF16, kind="ExternalInput")
    mbias = None
    if mode == "dense":
        mbias = nc.dram_tensor("mbias", [S, S], F32, kind="ExternalInput")
    out = nc.dram_tensor("out", [S, DIM], F16, kind="ExternalOutput")

    def dram3(t, d0, nd, s0, ns):
        """3D AP over a [DIM, S] dram tensor: nd chunks of 128 rows
        starting at row d0, ns cols from s0. Lands in an SBUF tile
        [128, nd*ns] chunk-major."""
        return bass.AP(tensor=t[0:1, 0:1].tensor, offset=d0 * S + s0,
                       ap=[[S, 128], [128 * S, nd], [1, ns]])

    with TileContext(nc) as tc:
        with (
            tc.tile_pool(name="consts", bufs=1) as consts,
            tc.tile_pool(name="w", bufs=1) as wpool,
            tc.tile_pool(name="xs", bufs=1) as xs,
            tc.tile_pool(name="acts", bufs=1) as acts,
            tc.tile_pool(name="qx", bufs=2) as qxp,
            tc.tile_pool(name="kx", bufs=2) as kxp,
            tc.tile_pool(name="vx", bufs=2) as vxp,
            tc.tile_pool(name="an", bufs=3) as anp,
            tc.tile_pool(name="exp", bufs=3) as expp,
            tc.tile_pool(name="nm", bufs=2) as nmp,
            tc.tile_pool(name="ob", bufs=2) as obp,
            tc.tile_pool(name="mbp", bufs=2) as mbp,
            tc.tile_pool(name="sc", bufs=2, space="PSUM") as scp,
            tc.tile_pool(name="at", bufs=1, space="PSUM") as atp,
            tc.tile_pool(name="acc", bufs=2, space="PSUM") as accp,
        ):
            # ---- constants ----
            tri_t = consts.tile([128, 128], BF16, tag="tri")
            nc.gpsimd.dma_start(out=tri_t[:, :], in_=tri[:, :])
            id_t = consts.tile([128, 128], BF16, tag="id")
            nc.gpsimd.dma_start(out=id_t[:, :], in_=ident[:, :])
            bq_t = consts.tile([128, 4], F32, tag="bq")
            nc.gpsimd.dma_start(
                out=bq_t[:, :],
                in_=bass.AP(tensor=bq[0:1].tensor, offset=0,
                            ap=[[1, 128], [128, 4]]))
            bkv_t = consts.tile([128, 2], F32, tag="bkv")
            nc.gpsimd.dma_start(
                out=bkv_t[:, :],
                in_=bass.AP(tensor=bkv[0:1].tensor, offset=0,
                            ap=[[1, 128], [128, 2]]))

            # ---- weights: single bulk DMA each, already bf16 ----
            # (wq/wkv issued here; wo deferred until after the first input
            # loads so GEMM1(0) isn't stuck behind 2MB it doesn't need)
            wq_t = wpool.tile([128, NDC * CQ], BF16, tag="wq")
            wkv_t = wpool.tile([128, NDC * 2 * CK], BF16, tag="wkv")
            wo_t = wpool.tile([128, 4 * DIM], BF16, tag="wo")

            def load_wq_wkv():
                nc.gpsimd.dma_start(
                    out=wq_t[:, :],
                    in_=bass.AP(tensor=wq[0:1, 0:1].tensor, offset=0,
                                ap=[[CQ, 128], [128 * CQ, NDC], [1, CQ]]))
                nc.gpsimd.dma_start(
                    out=wkv_t[:, :],
                    in_=bass.AP(tensor=wkv[0:1, 0:1].tensor, offset=0,
                                ap=[[2 * CK, 128], [128 * 2 * CK, NDC],
                                    [1, 2 * CK]]))

            def load_wo():
                nc.gpsimd.dma_start(
                    out=wo_t[:, :],
                    in_=bass.AP(tensor=wo[0:1, 0:1].tensor, offset=0,
                                ap=[[DIM, 128], [128 * DIM, 4], [1, DIM]]))

            def wq_ap(dc, cc):
                return wq_t[:, dc * CQ + cc * 128:dc * CQ + (cc + 1) * 128]

            def wk_ap(dc):
                return wkv_t[:, dc * 2 * CK:dc * 2 * CK + CK]

            def wv_ap(dc):
                return wkv_t[:, dc * 2 * CK + CK:(dc + 1) * 2 * CK]

            def wo_ap(cc, ec):
                return wo_t[:, cc * DIM + ec * 512:cc * DIM + (ec + 1) * 512]

            # ---- persistent activations ----
            # layout: qxT_ss[ss][cc] [128, 512]: partitions 0:64 = head cc
            # (kv group 0), 64:128 = head cc+4 (kv group 1). Host permutes
            # Wq cols / Wo rows to match.
            vx1 = [acts.tile([128, 130], BF16, tag=f"vp{jb}", name=f"vp{jb}")
                   for jb in range(S // 128)]
            qxT = [[None] * 4 for _ in range(NSS)]
            kxT = [None] * NSS
            attnT = [[None] * 4 for _ in range(NSS)]

            # input stream tiles (one ss in flight per tensor)
            def load_inputs(ss):
                # 3 bulk DMAs; single-buffered tiles: by the time load(ss)
                # is issued, GEMM1(ss-1) has consumed the previous content.
                s0 = ss * 512
                xq = xs.tile([128, NDC * 512], BF16, tag="xq", bufs=1,
                             name=f"xq{ss}")
                nc.gpsimd.dma_start(out=xq[:, :],
                                    in_=dram3(qT, 0, NDC, s0, 512))
                xk = xs.tile([128, NDC * 512], BF16, tag="xk", bufs=1,
                             name=f"xk{ss}")
                nc.gpsimd.dma_start(out=xk[:, :],
                                    in_=dram3(kT, 0, NDC, s0, 512))
                xv = xs.tile([128, NDC * 512], BF16, tag="xv", bufs=1,
                             name=f"xv{ss}")
                nc.gpsimd.dma_start(out=xv[:, :],
                                    in_=dram3(vT, 0, NDC, s0, 512))
                return xq, xk, xv

            def gemm1_closures(ss, xq, xk, xv):
                """Returns a list of closures, each issuing one projection
                chain for chunk ss."""
                s0 = ss * 512
                cls = []

                def q_chain(cc):
                    def run():
                        ps = accp.tile([128, 512], F32, tag="acc",
                                       name=f"g1q{ss}{cc}")
                        for dc in range(NDC):
                            nc.tensor.matmul(
                                ps[:, :], wq_ap(dc, cc),
                                xq[:, dc * 512:(dc + 1) * 512],
                                start=(dc == 0), stop=(dc == NDC - 1))
                        if mode == "causal":
                            t = qxp.tile([128, 512], BF16, tag=f"qx{cc}",
                                         name=f"qx{ss}{cc}")
                        else:
                            # flat schedule: all chunks' qxT live at once
                            t = qxp.tile([128, 512], BF16,
                                         tag=f"qx{ss}{cc}", bufs=1,
                                         name=f"qx{ss}{cc}")
                        nc.vector.tensor_scalar_add(t[:, :], ps[:, :],
                                                    bq_t[:, cc:cc + 1])
                        qxT[ss][cc] = t
                    return run

                for cc in range(4):
                    cls.append(q_chain(cc))

                def k_chain():
                    ps = accp.tile([128, 512], F32, tag="acc",
                                   name=f"g1k{ss}")
                    for dc in range(NDC):
                        nc.tensor.matmul(ps[:, :], wk_ap(dc),
                                         xk[:, dc * 512:(dc + 1) * 512],
                                         start=(dc == 0),
                                         stop=(dc == NDC - 1))
                    # kxT persists across the whole kernel (attention at
                    # chunk ss reads all chunks <= ss) -> per-ss tag.
                    t = kxp.tile([128, 512], BF16, tag=f"kx{ss}", bufs=1,
                                 name=f"kx{ss}")
                    nc.vector.tensor_scalar_add(t[:, :], ps[:, :],
                                                bkv_t[:, 0:1])
                    kxT[ss] = t
                cls.append(k_chain)

                def v_chain():
                    ps = accp.tile([128, 512], F32, tag="acc",
                                   name=f"g1v{ss}")
                    for dc in range(NDC):
                        nc.tensor.matmul(ps[:, :], wv_ap(dc),
                                         xv[:, dc * 512:(dc + 1) * 512],
                                         start=(dc == 0),
                                         stop=(dc == NDC - 1))
                    vxs = vxp.tile([128, 512], BF16, tag="vxT",
                                   name=f"vxT{ss}")
                    nc.vector.tensor_scalar_add(vxs[:, :], ps[:, :],
                                                bkv_t[:, 1:2])
                    # transpose to natural [j, c] and pack into vx1 with
                    # ones columns (flash denominator trick)
                    vtp = accp.tile([128, 512], BF16, tag="acc",
                                    name=f"vtp{ss}")
                    for sc in range(4):
                        nc.tensor.transpose(
                            vtp[:, sc * 128:(sc + 1) * 128],
                            vxs[:, sc * 128:(sc + 1) * 128], id_t[:, :])
                    for sc in range(4):
                        vx = vx1[ss * 4 + sc]
                        nc.vector.tensor_copy(
                            vx[:, 0:64], vtp[:, sc * 128:sc * 128 + 64])
                        nc.vector.tensor_copy(
                            vx[:, 65:129],
                            vtp[:, sc * 128 + 64:(sc + 1) * 128])
                        nc.vector.memset(vx[:, 64:65], 1.0)
                        nc.vector.memset(vx[:, 129:130], 1.0)
                cls.append(v_chain)
                return cls

            def gemm4_closures(ss):
                """Output projection for chunk ss (reads attnT[ss])."""
                s0 = ss * 512
                cls = []

                def one(sc):
                    def run():
                        i0 = s0 + sc * 128
                        ob = obp.tile([128, DIM], F16, tag="ob",
                                      name=f"ob{ss}{sc}")
                        for ec in range(4):
                            g4 = accp.tile([128, 512], F32, tag="acc",
                                           name=f"g4{ss}{sc}{ec}")
                            for cc2 in range(4):
                                nc.tensor.matmul(
                                    g4[:, :],
                                    attnT[ss][cc2][:, sc * 128:(sc + 1) * 128],
                                    wo_ap(cc2, ec),
                                    start=(cc2 == 0), stop=(cc2 == 3))
                            nc.vector.tensor_copy(
                                ob[:, ec * 512:(ec + 1) * 512], g4[:, :])
                        nc.sync.dma_start(out=out[i0:i0 + 128, :],
                                          in_=ob[:, :])
                    return run

                for sc in range(4):
                    cls.append(one(sc))
                return cls

            def pair_loop(ss, bg):
                """Attention for i-chunk ss, all 4 head pairs; injects
                background closures from bg between j-steps, spread evenly
                over the chunk's injection slots."""
                s0 = ss * 512
                njb = 4 * (ss + 1) if mode == "causal" else S // 128
                nslots = 4 * njb
                stride = max(1, nslots // max(1, len(bg)))
                slot = [0]

                def maybe_inject():
                    slot[0] += 1
                    if bg and slot[0] % stride == 0:
                        bg.pop(0)()
                for cc in range(4):
                    atl = atp.tile([65, 512], F32, tag="atl",
                                   name=f"atl{ss}{cc}")
                    ath = atp.tile([65, 512], F32, tag="ath",
                                   name=f"ath{ss}{cc}")
                    for jb in range(njb):
                        j0 = jb * 128
                        off = max(0, j0 - s0) if mode == "causal" else 0
                        N = 512 - off
                        kx = kxT[jb // 4]
                        kl = kx[0:64, (jb % 4) * 128:(jb % 4 + 1) * 128]
                        kh = kx[64:128, (jb % 4) * 128:(jb % 4 + 1) * 128]
                        sct = scp.tile([128, 1024], F32, tag="sc",
                                       name=f"sc{ss}{cc}{jb}")
                        nc.tensor.matmul(
                            sct[:, 0:N], kl,
                            qxT[ss][cc][0:64, off:512],
                            start=True, stop=True)
                        nc.tensor.matmul(
                            sct[:, 512:512 + N], kh,
                            qxT[ss][cc][64:128, off:512],
                            start=True, stop=True)
                        if mode == "dense":
                            mb = mbp.tile([128, 512], F32, tag="mb",
                                          name=f"mb{ss}{cc}{jb}")
                            nc.gpsimd.dma_start(
                                out=mb[:, :N],
                                in_=mbias[j0:j0 + 128, s0 + off:s0 + 512])
                            nc.vector.tensor_tensor(
                                sct[:, 0:N], sct[:, 0:N], mb[:, :N],
                                ALU.add)
                            nc.vector.tensor_tensor(
                                sct[:, 512:512 + N], sct[:, 512:512 + N],
                                mb[:, :N], ALU.add)
                        ex = expp.tile([128, 1024], BF16, tag="ex",
                                       name=f"ex{ss}{cc}{jb}")
                        if off == 0:
                            nc.scalar.activation(ex[:, :], sct[:, :],
                                                 AF.Exp, scale=0.125)
                        else:
                            nc.scalar.activation(ex[:, 0:N], sct[:, 0:N],
                                                 AF.Exp, scale=0.125)
                            nc.scalar.activation(ex[:, 512:512 + N],
                                                 sct[:, 512:512 + N],
                                                 AF.Exp, scale=0.125)
                        if mode == "causal" and j0 >= s0:
                            nc.vector.tensor_tensor(
                                ex[:, 0:128], ex[:, 0:128], tri_t[:, :],
                                ALU.mult)
                            nc.vector.tensor_tensor(
                                ex[:, 512:640], ex[:, 512:640], tri_t[:, :],
                                ALU.mult)
                        nc.tensor.matmul(
                            atl[:, off:512], vx1[jb][:, 0:65], ex[:, 0:N],
                            start=(jb == 0), stop=(jb == njb - 1))
                        nc.tensor.matmul(
                            ath[:, off:512], vx1[jb][:, 65:130],
                            ex[:, 512:512 + N],
                            start=(jb == 0), stop=(jb == njb - 1))
                        maybe_inject()
                    # ---- normalize both heads of the pair ----
                    # reciprocal of denominator row, partition-broadcast on
                    # GPSIMD (SBUF->SBUF, keeps PE out of the path), then
                    # one DVE mult per head (PSUM x SBUF -> SBUF).
                    att = anp.tile([128, 512], BF16, tag=f"an{cc}",
                                   name=f"an{ss}{cc}")
                    for hi, at in ((0, atl), (1, ath)):
                        # approx-recip mis-reads PSUM at partition 64;
                        # stage the denominator row through SBUF first.
                        den = nmp.tile([1, 512], F32, tag="den",
                                       name=f"den{ss}{cc}{hi}")
                        nc.vector.tensor_copy(den[0:1, :], at[64:65, :])
                        nmf = nmp.tile([1, 512], F32, tag="nmf",
                                       name=f"nmf{ss}{cc}{hi}")
                        nc.vector.reciprocal_approx_fast(nmf[0:1, :],
                                                         den[0:1, :])
                        nmb = nmp.tile([64, 512], F32, tag="nmb",
                                       name=f"nmb{ss}{cc}{hi}")
                        nc.gpsimd.partition_broadcast(nmb[:, :],
                                                      nmf[0:1, :])
                        nc.vector.tensor_tensor(
                            att[hi * 64:(hi + 1) * 64, :], at[0:64, :],
                            nmb[:, :], ALU.mult)
                    attnT[ss][cc] = att

            # ================= main schedule =================
            if mode == "causal":
                # software-pipelined: GEMM1(ss+1) and (two-chunk-deferred)
                # GEMM4 chains are injected into chunk ss's attention loop
                # to fill PE while ACT computes exp. Later chunks have more
                # ACT work (bigger njb), so GEMM4 lands there.
                xin = load_inputs(0)
                load_wq_wkv()
                for c in gemm1_closures(0, *xin):
                    c()
                load_wo()
                xin = load_inputs(1)
                # per-chunk background work: ss0: G1(1); ss1: G1(2);
                # ss2: G1(3)+G4(0); ss3: G4(1)+G4(2); epilogue: G4(3)
                for ss in range(NSS):
                    bg = []
                    if ss + 1 < NSS:
                        bg.extend(gemm1_closures(ss + 1, *xin))
                    if ss >= 2:
                        bg.extend(gemm4_closures(ss - 2))
                    if ss == NSS - 1:
                        bg.extend(gemm4_closures(ss - 1))
                    pair_loop(ss, bg)
                    for c in bg:
                        c()
                    if ss + 2 < NSS:
                        xin = load_inputs(ss + 2)
                for c in gemm4_closures(NSS - 1):
                    c()
            else:
                # non-causal: attention at any chunk reads ALL k/v chunks,
                # so all projections must complete first (flat schedule).
                for ss in range(NSS):
                    xin = load_inputs(ss)
                    for c in gemm1_closures(ss, *xin):
                        c()
                for ss in range(NSS):
                    bg = gemm4_closures(ss - 1) if ss > 0 else []
                    pair_loop(ss, bg)
                    for c in bg:
                        c()
                for c in gemm4_closures(NSS - 1):
                    c()

    nc.finalize()
    return nc


_CACHE = {}


def _get_nc(mode):
    if mode not in _CACHE:
        _CACHE[mode] = build_nc(mode)
    return _CACHE[mode]


def _to_bf16(x):
    """Fast float32 -> bfloat16 with round-to-nearest-even."""
    x = np.ascontiguousarray(x, np.float32)
    u = x.view(np.uint32)
    r = ((u + 0x7FFF + ((u >> 16) & 1)) >> 16).astype(np.uint16)
    return r.view(ml_dtypes.bfloat16)


def kernel(q, k, v, mask, Wq, bq, Wk, bk, Wv, bv, Wo, bo):
    q = np.asarray(q, np.float32)
    k = np.asarray(k, np.float32)
    v = np.asarray(v, np.float32)
    mask = np.asarray(mask)
    Wq = np.asarray(Wq, np.float32)
    Wk = np.asarray(Wk, np.float32)
    Wv = np.asarray(Wv, np.float32)
    Wo = np.asarray(Wo, np.float32)
    bq = np.asarray(bq, np.float32)
    bk = np.asarray(bk, np.float32)
    bv = np.asarray(bv, np.float32)
    bo = np.asarray(bo, np.float32)

    m = mask.astype(np.float64)
    if np.array_equal(m, np.tril(np.ones((S, S)))):
        mode = "causal"
    elif np.all(m == 1):
        mode = "none"
    else:
        mode = "dense"

    nc = _get_nc(mode)
    tri_np = np.triu(np.ones((128, 128))).astype(ml_dtypes.bfloat16)
    id_np = np.eye(128).astype(ml_dtypes.bfloat16)

    # transposed bf16 copies of q/k/v, shared across the 4 cores per batch
    qT = [_to_bf16(q[b].T) for b in range(B)]
    kT = [_to_bf16(k[b].T) for b in range(B)]
    vT = [_to_bf16(v[b].T) for b in range(B)]

    # On-chip layout places local q head h in tile h%4 at partition
    # (h//4)*64 so q/k partition bases match in the scores matmul. Permute
    # Wq columns / Wo rows / bq accordingly: tile cc holds heads (cc, cc+4).
    head_perm = [h for cc in range(4) for h in (cc, cc + 4)]
    col_perm = np.concatenate(
        [np.arange(h * HD, (h + 1) * HD) for h in head_perm])

    in_maps = []
    for core in range(NCORES):
        b, kb = core // KVSH, core % KVSH
        wq_sh = Wq[:, kb * CQ:(kb + 1) * CQ][:, col_perm]
        wo_sh = Wo[kb * CQ:(kb + 1) * CQ, :][col_perm, :]
        bq_sh = bq[kb * CQ:(kb + 1) * CQ][col_perm]
        wkv_sh = np.concatenate(
            [Wk[:, kb * CK:(kb + 1) * CK], Wv[:, kb * CK:(kb + 1) * CK]],
            axis=1)
        bkv_sh = np.concatenate(
            [bk[kb * CK:(kb + 1) * CK], bv[kb * CK:(kb + 1) * CK]])
        im = {
            "qT": qT[b],
            "kT": kT[b],
            "vT": vT[b],
            "wq": _to_bf16(wq_sh),
            "wkv": _to_bf16(wkv_sh),
            "wo": _to_bf16(wo_sh),
            "bq": np.ascontiguousarray(bq_sh),
            "bkv": bkv_sh,
            "tri": tri_np,
            "ident": id_np,
        }
        if mode == "dense":
            with np.errstate(divide="ignore"):
                bias = -(1.0 / mask.astype(np.float32) + 1.0)
            im["mbias"] = np.ascontiguousarray(bias.T * 8.0)
        in_maps.append(im)

    res = run_bass_kernel_spmd(nc, in_maps, core_ids=list(range(NCORES)))
    outs = [r["out"] for r in res.results]
    full = np.empty((B, S, DIM), np.float32)
    for b in range(B):
        acc = outs[b * KVSH].astype(np.float32)
        for kb in range(1, KVSH):
            acc = acc + outs[b * KVSH + kb]
        full[b] = acc + bo[None, :]
    return full


# revision 19
# speedup vs baseline: 2.4087x; 1.0068x over previous
"""Grouped-query attention (GQA) Trainium2 Bass kernel, v2.

Problem: B=2, S=2048, DIM=2048, HQ=32, HKV=8, HEAD_DIM=64, causal mask.
Sharding: 8 cores = 2 (batch) x 4 (kv-head groups). Core c handles batch
c//4 and kv-block c%4 (2 kv heads, 8 q heads). Wq/Wk/Wv sharded
column-wise, Wo row-wise; each core writes a partial [S, DIM] fp16
output; host sums the 4 partials per batch and adds bo.

v2 changes vs v1:
  - q/k/v pre-transposed and pre-cast to bf16 on host -> no on-chip
    casts/PE transposes; weights pre-cast to bf16 (halves DMA bytes).
  - Bulk 3D-AP DMAs (one per tensor per seq chunk) instead of 48 small
    ones.
  - Score matmuls for the two kv-head groups issued back-to-back at
    partition bases 0/64 -> concurrent PE row-tiles (2x on K=64 work).
  - exp over [128,1024] (both heads of a pair) per j-block.
  - Denominator: reciprocal_approx_fast + PE outer-product broadcast
    (no DRAM round-trip, no slow DVE reciprocal).
  - GEMM1/GEMM4 evacuations on DVE (tensor_scalar add for bias), exp
    exclusively on ACT.
  - GEMM4(ss-1) and GEMM1(ss+1) chains interleaved into the attention
    pair loop to fill PE while ACT computes exp.
  - fp16 partial outputs (halves output DMA).
"""

import numpy as np
import ml_dtypes

import concourse.bass as bass
import concourse.mybir as mybir
from concourse import bacc
from concourse.tile import TileContext
from concourse.bass_utils import run_bass_kernel_spmd

F32 = mybir.dt.float32
F16 = mybir.dt.float16
BF16 = mybir.dt.bfloat16
AF = mybir.ActivationFunctionType
ALU = mybir.AluOpType

B, S, DIM = 2, 2048, 2048
HQ, HKV, HD = 32, 8, 64
GROUP = HQ // HKV              # 4
NCORES = 8
KVSH = 4                       # kv-blocks (shards) per batch
CQ = (HQ // KVSH) * HD         # 512 q-proj cols per core (8 heads)
CK = (HKV // KVSH) * HD        # 128 kv-proj cols per core (2 heads)
NDC = DIM // 128               # 16 contraction chunks
NSS = S // 512                 # 4 sequence chunks of 512


def build_nc(mode="causal"):
    nc = bacc.Bacc("TRN2", target_bir_lowering=False)

    qT = nc.dram_tensor("qT", [DIM, S], BF16, kind="ExternalInput")
    kT = nc.dram_tensor("kT", [DIM, S], BF16, kind="ExternalInput")
    vT = nc.dram_tensor("vT", [DIM, S], BF16, kind="ExternalInput")
    wq = nc.dram_tensor("wq", [DIM, CQ], BF16, kind="ExternalInput")
    wkv = nc.dram_tensor("wkv", [DIM, 2 * CK], BF16, kind="ExternalInput")
    wo = nc.dram_tensor("wo", [CQ, DIM], BF16, kind="ExternalInput")
    bq = nc.dram_tensor("bq", [CQ], F32, kind="ExternalInput")
    bkv = nc.dram_tensor("bkv", [2 * CK], F32, kind="ExternalInput")
    tri = nc.dram_tensor("tri", [128, 128], BF16, kind="ExternalInput")
    ident = nc.dram_tensor("ident", [128, 128], BF16, kind="ExternalInput")
    mbias = None
    if mode == "dense":
        mbias = nc.dram_tensor("mbias", [S, S], F32, kind="ExternalInput")
    out = nc.dram_tensor("out", [S, DIM], F16, kind="ExternalOutput")

    def dram3(t, d0, nd, s0, ns):
        """3D AP over a [DIM, S] dram tensor: nd chunks of 128 rows
        starting at row d0, ns cols from s0. Lands in an SBUF tile
        [128, nd*ns] chunk-major."""
        return bass.AP(tensor=t[0:1, 0:1].tensor, offset=d0 * S + s0,
                       ap=[[S, 128], [128 * S, nd], [1, ns]])

    with TileContext(nc) as tc:
        with (
            tc.tile_pool(name="consts", bufs=1) as consts,
            tc.tile_pool(name="w", bufs=1) as wpool,
            tc.tile_pool(name="xs", bufs=1) as xs,
            tc.tile_pool(name="acts", bufs=1) as acts,
            tc.tile_pool(name="qx", bufs=2) as qxp,
            tc.tile_pool(name="kx", bufs=2) as kxp,
            tc.tile_pool(name="vx", bufs=2) as vxp,
            tc.tile_pool(name="an", bufs=3) as anp,
            tc.tile_pool(name="exp", bufs=3) as expp,
            tc.tile_pool(name="nm", bufs=2) as nmp,
            tc.tile_pool(name="ob", bufs=2) as obp,
            tc.tile_pool(name="mbp", bufs=2) as mbp,
            tc.tile_pool(name="sc", bufs=2, space="PSUM") as scp,
            tc.tile_pool(name="at", bufs=1, space="PSUM") as atp,
            tc.tile_pool(name="acc", bufs=2, space="PSUM") as accp,
        ):
            # ---- constants (tiny; on sync queue so they don't delay the
            # bulk input/weight loads below) ----
            tri_t = consts.tile([128, 128], BF16, tag="tri")
            nc.sync.dma_start(out=tri_t[:, :], in_=tri[:, :])
            id_t = consts.tile([128, 128], BF16, tag="id")
            nc.sync.dma_start(out=id_t[:, :], in_=ident[:, :])
            bq_t = consts.tile([128, 4], F32, tag="bq")
            nc.sync.dma_start(
                out=bq_t[:, :],
                in_=bass.AP(tensor=bq[0:1].tensor, offset=0,
                            ap=[[1, 128], [128, 4]]))
            bkv_t = consts.tile([128, 2], F32, tag="bkv")
            nc.sync.dma_start(
                out=bkv_t[:, :],
                in_=bass.AP(tensor=bkv[0:1].tensor, offset=0,
                            ap=[[1, 128], [128, 2]]))

            # ---- weights: single bulk DMA each, already bf16 ----
            # (wq/wkv issued here; wo deferred until after the first input
            # loads so GEMM1(0) isn't stuck behind 2MB it doesn't need)
            wq_t = wpool.tile([128, NDC * CQ], BF16, tag="wq")
            wkv_t = wpool.tile([128, NDC * 2 * CK], BF16, tag="wkv")
            wo_t = wpool.tile([128, 4 * DIM], BF16, tag="wo")

            def load_wq_wkv():
                # scalar queue: runs in parallel with the xq load on the
                # gpsimd queue, so GEMM1(0) starts ~15us earlier
                nc.scalar.dma_start(
                    out=wq_t[:, :],
                    in_=bass.AP(tensor=wq[0:1, 0:1].tensor, offset=0,
                                ap=[[CQ, 128], [128 * CQ, NDC], [1, CQ]]))
                nc.scalar.dma_start(
                    out=wkv_t[:, :],
                    in_=bass.AP(tensor=wkv[0:1, 0:1].tensor, offset=0,
                                ap=[[2 * CK, 128], [128 * 2 * CK, NDC],
                                    [1, 2 * CK]]))

            def load_wo():
                nc.sync.dma_start(
                    out=wo_t[:, :],
                    in_=bass.AP(tensor=wo[0:1, 0:1].tensor, offset=0,
                                ap=[[DIM, 128], [128 * DIM, 4], [1, DIM]]))

            def wq_ap(dc, cc):
                return wq_t[:, dc * CQ + cc * 128:dc * CQ + (cc + 1) * 128]

            def wk_ap(dc):
                return wkv_t[:, dc * 2 * CK:dc * 2 * CK + CK]

            def wv_ap(dc):
                return wkv_t[:, dc * 2 * CK + CK:(dc + 1) * 2 * CK]

            def wo_ap(cc, ec):
                return wo_t[:, cc * DIM + ec * 512:cc * DIM + (ec + 1) * 512]

            # ---- persistent activations ----
            # layout: qxT_ss[ss][cc] [128, 512]: partitions 0:64 = head cc
            # (kv group 0), 64:128 = head cc+4 (kv group 1). Host permutes
            # Wq cols / Wo rows to match.
            vx1 = [acts.tile([128, 130], BF16, tag=f"vp{jb}", name=f"vp{jb}")
                   for jb in range(S // 128)]
            qxT = [[None] * 4 for _ in range(NSS)]
            kxT = [None] * NSS
            attnT = [[None] * 4 for _ in range(NSS)]

            # input stream tiles (one ss in flight per tensor)
            def load_inputs(ss):
                # 3 bulk DMAs; single-buffered tiles: by the time load(ss)
                # is issued, GEMM1(ss-1) has consumed the previous content.
                s0 = ss * 512
                xq = xs.tile([128, NDC * 512], BF16, tag="xq", bufs=1,
                             name=f"xq{ss}")
                nc.gpsimd.dma_start(out=xq[:, :],
                                    in_=dram3(qT, 0, NDC, s0, 512))
                xk = xs.tile([128, NDC * 512], BF16, tag="xk", bufs=1,
                             name=f"xk{ss}")
                nc.gpsimd.dma_start(out=xk[:, :],
                                    in_=dram3(kT, 0, NDC, s0, 512))
                xv = xs.tile([128, NDC * 512], BF16, tag="xv", bufs=1,
                             name=f"xv{ss}")
                nc.gpsimd.dma_start(out=xv[:, :],
                                    in_=dram3(vT, 0, NDC, s0, 512))
                return xq, xk, xv

            def gemm1_closures(ss, xq, xk, xv):
                """Returns a list of closures, each issuing one projection
                chain for chunk ss."""
                s0 = ss * 512
                cls = []

                def q_chain(cc):
                    def run():
                        ps = accp.tile([128, 512], F32, tag="acc",
                                       name=f"g1q{ss}{cc}")
                        for dc in range(NDC):
                            nc.tensor.matmul(
                                ps[:, :], wq_ap(dc, cc),
                                xq[:, dc * 512:(dc + 1) * 512],
                                start=(dc == 0), stop=(dc == NDC - 1))
                        if mode == "causal":
                            t = qxp.tile([128, 512], BF16, tag=f"qx{cc}",
                                         name=f"qx{ss}{cc}")
                        else:
                            # flat schedule: all chunks' qxT live at once
                            t = qxp.tile([128, 512], BF16,
                                         tag=f"qx{ss}{cc}", bufs=1,
                                         name=f"qx{ss}{cc}")
                        nc.vector.tensor_scalar_add(t[:, :], ps[:, :],
                                                    bq_t[:, cc:cc + 1])
                        qxT[ss][cc] = t
                    return run

                for cc in range(4):
                    cls.append(q_chain(cc))

                def k_chain():
                    ps = accp.tile([128, 512], F32, tag="acc",
                                   name=f"g1k{ss}")
                    for dc in range(NDC):
                        nc.tensor.matmul(ps[:, :], wk_ap(dc),
                                         xk[:, dc * 512:(dc + 1) * 512],
                                         start=(dc == 0),
                                         stop=(dc == NDC - 1))
                    # kxT persists across the whole kernel (attention at
                    # chunk ss reads all chunks <= ss) -> per-ss tag.
                    t = kxp.tile([128, 512], BF16, tag=f"kx{ss}", bufs=1,
                                 name=f"kx{ss}")
                    nc.vector.tensor_scalar_add(t[:, :], ps[:, :],
                                                bkv_t[:, 0:1])
                    kxT[ss] = t
                cls.append(k_chain)

                def v_chain():
                    ps = accp.tile([128, 512], F32, tag="acc",
                                   name=f"g1v{ss}")
                    for dc in range(NDC):
                        nc.tensor.matmul(ps[:, :], wv_ap(dc),
                                         xv[:, dc * 512:(dc + 1) * 512],
                                         start=(dc == 0),
                                         stop=(dc == NDC - 1))
                    vxs = vxp.tile([128, 512], BF16, tag="vxT",
                                   name=f"vxT{ss}")
                    nc.vector.tensor_scalar_add(vxs[:, :], ps[:, :],
                                                bkv_t[:, 1:2])
                    # transpose to natural [j, c] and pack into vx1 with
                    # ones columns (flash denominator trick)
                    vtp = accp.tile([128, 512], BF16, tag="acc",
                                    name=f"vtp{ss}")
                    for sc in range(4):
                        nc.tensor.transpose(
                            vtp[:, sc * 128:(sc + 1) * 128],
                            vxs[:, sc * 128:(sc + 1) * 128], id_t[:, :])
                    for sc in range(4):
                        vx = vx1[ss * 4 + sc]
                        nc.vector.tensor_copy(
                            vx[:, 0:64], vtp[:, sc * 128:sc * 128 + 64])
                        nc.vector.tensor_copy(
                            vx[:, 65:129],
                            vtp[:, sc * 128 + 64:(sc + 1) * 128])
                        nc.vector.memset(vx[:, 64:65], 1.0)
                        nc.vector.memset(vx[:, 129:130], 1.0)
                cls.append(v_chain)
                return cls

            def gemm4_closures(ss):
                """Output projection for chunk ss (reads attnT[ss])."""
                s0 = ss * 512
                cls = []

                def one(sc):
                    def run():
                        i0 = s0 + sc * 128
                        ob = obp.tile([128, DIM], F16, tag="ob",
                                      name=f"ob{ss}{sc}")
                        for ec in range(4):
                            g4 = accp.tile([128, 512], F32, tag="acc",
                                           name=f"g4{ss}{sc}{ec}")
                            for cc2 in range(4):
                                nc.tensor.matmul(
                                    g4[:, :],
                                    attnT[ss][cc2][:, sc * 128:(sc + 1) * 128],
                                    wo_ap(cc2, ec),
                                    start=(cc2 == 0), stop=(cc2 == 3))
                            nc.vector.tensor_copy(
                                ob[:, ec * 512:(ec + 1) * 512], g4[:, :])
                        nc.sync.dma_start(out=out[i0:i0 + 128, :],
                                          in_=ob[:, :])
                    return run

                for sc in range(4):
                    cls.append(one(sc))
                return cls

            def pair_loop(ss, bg):
                """Attention for i-chunk ss, all 4 head pairs; injects
                background closures from bg between j-steps, spread evenly
                over the chunk's injection slots."""
                s0 = ss * 512
                njb = 4 * (ss + 1) if mode == "causal" else S // 128
                nslots = 4 * njb
                stride = max(1, nslots // max(1, len(bg)))
                slot = [0]

                def maybe_inject():
                    slot[0] += 1
                    if bg and slot[0] % stride == 0:
                        bg.pop(0)()
                for cc in range(4):
                    atl = atp.tile([65, 512], F32, tag="atl",
                                   name=f"atl{ss}{cc}")
                    ath = atp.tile([65, 512], F32, tag="ath",
                                   name=f"ath{ss}{cc}")
                    for jb in range(njb):
                        j0 = jb * 128
                        off = max(0, j0 - s0) if mode == "causal" else 0
                        N = 512 - off
                        kx = kxT[jb // 4]
                        kl = kx[0:64, (jb % 4) * 128:(jb % 4 + 1) * 128]
                        kh = kx[64:128, (jb % 4) * 128:(jb % 4 + 1) * 128]
                        sct = scp.tile([128, 1024], F32, tag="sc",
                                       name=f"sc{ss}{cc}{jb}")
                        nc.tensor.matmul(
                            sct[:, 0:N], kl,
                            qxT[ss][cc][0:64, off:512],
                            start=True, stop=True)
                        nc.tensor.matmul(
                            sct[:, 512:512 + N], kh,
                            qxT[ss][cc][64:128, off:512],
                            start=True, stop=True)
                        if mode == "dense":
                            mb = mbp.tile([128, 512], F32, tag="mb",
                                          name=f"mb{ss}{cc}{jb}")
                            nc.gpsimd.dma_start(
                                out=mb[:, :N],
                                in_=mbias[j0:j0 + 128, s0 + off:s0 + 512])
                            nc.vector.tensor_tensor(
                                sct[:, 0:N], sct[:, 0:N], mb[:, :N],
                                ALU.add)
                            nc.vector.tensor_tensor(
                                sct[:, 512:512 + N], sct[:, 512:512 + N],
                                mb[:, :N], ALU.add)
                        ex = expp.tile([128, 1024], BF16, tag="ex",
                                       name=f"ex{ss}{cc}{jb}")
                        if off == 0:
                            nc.scalar.activation(ex[:, :], sct[:, :],
                                                 AF.Exp, scale=0.125)
                        else:
                            nc.scalar.activation(ex[:, 0:N], sct[:, 0:N],
                                                 AF.Exp, scale=0.125)
                            nc.scalar.activation(ex[:, 512:512 + N],
                                                 sct[:, 512:512 + N],
                                                 AF.Exp, scale=0.125)
                        if mode == "causal" and j0 >= s0:
                            nc.vector.tensor_tensor(
                                ex[:, 0:128], ex[:, 0:128], tri_t[:, :],
                                ALU.mult)
                            nc.vector.tensor_tensor(
                                ex[:, 512:640], ex[:, 512:640], tri_t[:, :],
                                ALU.mult)
                        nc.tensor.matmul(
                            atl[:, off:512], vx1[jb][:, 0:65], ex[:, 0:N],
                            start=(jb == 0), stop=(jb == njb - 1))
                        nc.tensor.matmul(
                            ath[:, off:512], vx1[jb][:, 65:130],
                            ex[:, 512:512 + N],
                            start=(jb == 0), stop=(jb == njb - 1))
                        maybe_inject()
                    # ---- normalize both heads of the pair ----
                    # reciprocal of denominator row, partition-broadcast on
                    # GPSIMD (SBUF->SBUF, keeps PE out of the path), then
                    # one DVE mult per head (PSUM x SBUF -> SBUF).
                    att = anp.tile([128, 512], BF16, tag=f"an{cc}",
                                   name=f"an{ss}{cc}")
                    for hi, at in ((0, atl), (1, ath)):
                        # approx-recip mis-reads PSUM at partition 64;
                        # stage the denominator row through SBUF first.
                        den = nmp.tile([1, 512], F32, tag="den",
                                       name=f"den{ss}{cc}{hi}")
                        nc.vector.tensor_copy(den[0:1, :], at[64:65, :])
                        nmf = nmp.tile([1, 512], F32, tag="nmf",
                                       name=f"nmf{ss}{cc}{hi}")
                        nc.vector.reciprocal_approx_fast(nmf[0:1, :],
                                                         den[0:1, :])
                        nmb = nmp.tile([64, 512], F32, tag="nmb",
                                       name=f"nmb{ss}{cc}{hi}")
                        nc.gpsimd.partition_broadcast(nmb[:, :],
                                                      nmf[0:1, :])
                        nc.vector.tensor_tensor(
                            att[hi * 64:(hi + 1) * 64, :], at[0:64, :],
                            nmb[:, :], ALU.mult)
                    attnT[ss][cc] = att

            # ================= main schedule =================
            if mode == "causal":
                # software-pipelined: GEMM1(ss+1) and (two-chunk-deferred)
                # GEMM4 chains are injected into chunk ss's attention loop
                # to fill PE while ACT computes exp. Later chunks have more
                # ACT work (bigger njb), so GEMM4 lands there.
                xin = load_inputs(0)
                load_wq_wkv()
                for c in gemm1_closures(0, *xin):
                    c()
                load_wo()
                xin = load_inputs(1)
                # per-chunk background work: ss0: G1(1); ss1: G1(2);
                # ss2: G1(3)+G4(0); ss3: G4(1)+G4(2); epilogue: G4(3)
                for ss in range(NSS):
                    bg = []
                    if ss + 1 < NSS:
                        bg.extend(gemm1_closures(ss + 1, *xin))
                    if ss >= 2:
                        bg.extend(gemm4_closures(ss - 2))
                    if ss == NSS - 1:
                        bg.extend(gemm4_closures(ss - 1))
                    pair_loop(ss, bg)
                    for c in bg:
                        c()
                    if ss + 2 < NSS:
                        xin = load_inputs(ss + 2)
                for c in gemm4_closures(NSS - 1):
                    c()
            else:
                # non-causal: attention at any chunk reads ALL k/v chunks,
                # so all projections must complete first (flat schedule).
                for ss in range(NSS):
                    xin = load_inputs(ss)
                    for c in gemm1_closures(ss, *xin):
                        c()
                for ss in range(NSS):
                    bg = gemm4_closures(ss - 1) if ss > 0 else []
                    pair_loop(ss, bg)
                    for c in bg:
                        c()
                for c in gemm4_closures(NSS - 1):
                    c()

    nc.finalize()
    return nc


_CACHE = {}


def _get_nc(mode):
    if mode not in _CACHE:
        _CACHE[mode] = build_nc(mode)
    return _CACHE[mode]


def _to_bf16(x):
    """Fast float32 -> bfloat16 with round-to-nearest-even."""
    x = np.ascontiguousarray(x, np.float32)
    u = x.view(np.uint32)
    r = ((u + 0x7FFF + ((u >> 16) & 1)) >> 16).astype(np.uint16)
    return r.view(ml_dtypes.bfloat16)


def kernel(q, k, v, mask, Wq, bq, Wk, bk, Wv, bv, Wo, bo):
    q = np.asarray(q, np.float32)
    k = np.asarray(k, np.float32)
    v = np.asarray(v, np.float32)
    mask = np.asarray(mask)
    Wq = np.asarray(Wq, np.float32)
    Wk = np.asarray(Wk, np.float32)
    Wv = np.asarray(Wv, np.float32)
    Wo = np.asarray(Wo, np.float32)
    bq = np.asarray(bq, np.float32)
    bk = np.asarray(bk, np.float32)
    bv = np.asarray(bv, np.float32)
    bo = np.asarray(bo, np.float32)

    m = mask.astype(np.float64)
    if np.array_equal(m, np.tril(np.ones((S, S)))):
        mode = "causal"
    elif np.all(m == 1):
        mode = "none"
    else:
        mode = "dense"

    nc = _get_nc(mode)
    tri_np = np.triu(np.ones((128, 128))).astype(ml_dtypes.bfloat16)
    id_np = np.eye(128).astype(ml_dtypes.bfloat16)

    # transposed bf16 copies of q/k/v, shared across the 4 cores per batch
    qT = [_to_bf16(q[b].T) for b in range(B)]
    kT = [_to_bf16(k[b].T) for b in range(B)]
    vT = [_to_bf16(v[b].T) for b in range(B)]

    # On-chip layout places local q head h in tile h%4 at partition
    # (h//4)*64 so q/k partition bases match in the scores matmul. Permute
    # Wq columns / Wo rows / bq accordingly: tile cc holds heads (cc, cc+4).
    head_perm = [h for cc in range(4) for h in (cc, cc + 4)]
    col_perm = np.concatenate(
        [np.arange(h * HD, (h + 1) * HD) for h in head_perm])

    in_maps = []
    for core in range(NCORES):
        b, kb = core // KVSH, core % KVSH
        wq_sh = Wq[:, kb * CQ:(kb + 1) * CQ][:, col_perm]
        wo_sh = Wo[kb * CQ:(kb + 1) * CQ, :][col_perm, :]
        bq_sh = bq[kb * CQ:(kb + 1) * CQ][col_perm]
        wkv_sh = np.concatenate(
            [Wk[:, kb * CK:(kb + 1) * CK], Wv[:, kb * CK:(kb + 1) * CK]],
            axis=1)
        bkv_sh = np.concatenate(
            [bk[kb * CK:(kb + 1) * CK], bv[kb * CK:(kb + 1) * CK]])
        im = {
            "qT": qT[b],
            "kT": kT[b],
            "vT": vT[b],
            "wq": _to_bf16(wq_sh),
            "wkv": _to_bf16(wkv_sh),
            "wo": _to_bf16(wo_sh),
            "bq": np.ascontiguousarray(bq_sh),
            "bkv": bkv_sh,
            "tri": tri_np,
            "ident": id_np,
        }
        if mode == "dense":
            with np.errstate(divide="ignore"):
                bias = -(1.0 / mask.astype(np.float32) + 1.0)
            im["mbias"] = np.ascontiguousarray(bias.T * 8.0)
        in_maps.append(im)

    res = run_bass_kernel_spmd(nc, in_maps, core_ids=list(range(NCORES)))
    outs = [r["out"] for r in res.results]
    full = np.empty((B, S, DIM), np.float32)
    for b in range(B):
        acc = outs[b * KVSH].astype(np.float32)
        for kb in range(1, KVSH):
            acc = acc + outs[b * KVSH + kb]
        full[b] = acc + bo[None, :]
    return full


# revision 27
# speedup vs baseline: 2.4593x; 1.0210x over previous
"""Grouped-query attention (GQA) Trainium2 Bass kernel, v2.

Problem: B=2, S=2048, DIM=2048, HQ=32, HKV=8, HEAD_DIM=64, causal mask.
Sharding: 8 cores = 2 (batch) x 4 (kv-head groups). Core c handles batch
c//4 and kv-block c%4 (2 kv heads, 8 q heads). Wq/Wk/Wv sharded
column-wise, Wo row-wise; each core writes a partial [S, DIM] fp16
output; host sums the 4 partials per batch and adds bo.

v2 changes vs v1:
  - q/k/v pre-transposed and pre-cast to bf16 on host -> no on-chip
    casts/PE transposes; weights pre-cast to bf16 (halves DMA bytes).
  - Bulk 3D-AP DMAs (one per tensor per seq chunk) instead of 48 small
    ones.
  - Score matmuls for the two kv-head groups issued back-to-back at
    partition bases 0/64 -> concurrent PE row-tiles (2x on K=64 work).
  - exp over [128,1024] (both heads of a pair) per j-block.
  - Denominator: reciprocal_approx_fast + PE outer-product broadcast
    (no DRAM round-trip, no slow DVE reciprocal).
  - GEMM1/GEMM4 evacuations on DVE (tensor_scalar add for bias), exp
    exclusively on ACT.
  - GEMM4(ss-1) and GEMM1(ss+1) chains interleaved into the attention
    pair loop to fill PE while ACT computes exp.
  - fp16 partial outputs (halves output DMA).
"""

import numpy as np
import ml_dtypes

import concourse.bass as bass
import concourse.mybir as mybir
from concourse import bacc
from concourse.tile import TileContext
from concourse.bass_utils import run_bass_kernel_spmd

F32 = mybir.dt.float32
F16 = mybir.dt.float16
BF16 = mybir.dt.bfloat16
AF = mybir.ActivationFunctionType
ALU = mybir.AluOpType

B, S, DIM = 2, 2048, 2048
HQ, HKV, HD = 32, 8, 64
GROUP = HQ // HKV              # 4
NCORES = 8
KVSH = 4                       # kv-blocks (shards) per batch
CQ = (HQ // KVSH) * HD         # 512 q-proj cols per core (8 heads)
CK = (HKV // KVSH) * HD        # 128 kv-proj cols per core (2 heads)
NDC = DIM // 128               # 16 contraction chunks
NSS = S // 512                 # 4 sequence chunks of 512


def build_nc(mode="causal"):
    nc = bacc.Bacc("TRN2", target_bir_lowering=False)

    # All big operands are pre-swizzled on the host into the exact SBUF
    # layout [128 partitions, free], so every DMA is a plain 2D transfer
    # with multi-KB contiguous lines (near-peak HBM bandwidth).
    qT = nc.dram_tensor("qT", [128, NSS * NDC * 512], BF16,
                        kind="ExternalInput")
    kT = nc.dram_tensor("kT", [128, NSS * NDC * 512], BF16,
                        kind="ExternalInput")
    vT = nc.dram_tensor("vT", [128, NSS * NDC * 512], BF16,
                        kind="ExternalInput")
    wq = nc.dram_tensor("wq", [128, NDC * CQ], BF16, kind="ExternalInput")
    wkv = nc.dram_tensor("wkv", [128, NDC * 2 * CK], BF16,
                         kind="ExternalInput")
    wo = nc.dram_tensor("wo", [128, 4 * DIM], BF16, kind="ExternalInput")
    bq = nc.dram_tensor("bq", [CQ], F32, kind="ExternalInput")
    bkv = nc.dram_tensor("bkv", [2 * CK], F32, kind="ExternalInput")
    tri = nc.dram_tensor("tri", [128, 128], BF16, kind="ExternalInput")
    ident = nc.dram_tensor("ident", [128, 128], BF16, kind="ExternalInput")
    mbias = None
    if mode == "dense":
        mbias = nc.dram_tensor("mbias", [S, S], F32, kind="ExternalInput")
    out = nc.dram_tensor("out", [S, DIM], F16, kind="ExternalOutput")

    with TileContext(nc) as tc:
        with (
            tc.tile_pool(name="consts", bufs=1) as consts,
            tc.tile_pool(name="w", bufs=1) as wpool,
            tc.tile_pool(name="xs", bufs=1) as xs,
            tc.tile_pool(name="acts", bufs=1) as acts,
            tc.tile_pool(name="qx", bufs=2) as qxp,
            tc.tile_pool(name="kx", bufs=2) as kxp,
            tc.tile_pool(name="vx", bufs=2) as vxp,
            tc.tile_pool(name="an", bufs=3) as anp,
            tc.tile_pool(name="exp", bufs=3) as expp,
            tc.tile_pool(name="nm", bufs=2) as nmp,
            tc.tile_pool(name="ob", bufs=2) as obp,
            tc.tile_pool(name="mbp", bufs=2) as mbp,
            tc.tile_pool(name="sc", bufs=2, space="PSUM") as scp,
            tc.tile_pool(name="at", bufs=1, space="PSUM") as atp,
            tc.tile_pool(name="acc", bufs=2, space="PSUM") as accp,
        ):
            # ---- constants (tiny; on sync queue so they don't delay the
            # bulk input/weight loads below) ----
            tri_t = consts.tile([128, 128], BF16, tag="tri")
            nc.sync.dma_start(out=tri_t[:, :], in_=tri[:, :])
            id_t = consts.tile([128, 128], BF16, tag="id")
            nc.sync.dma_start(out=id_t[:, :], in_=ident[:, :])
            bq_t = consts.tile([128, 4], F32, tag="bq")
            nc.sync.dma_start(
                out=bq_t[:, :],
                in_=bass.AP(tensor=bq[0:1].tensor, offset=0,
                            ap=[[1, 128], [128, 4]]))
            bkv_t = consts.tile([128, 2], F32, tag="bkv")
            nc.sync.dma_start(
                out=bkv_t[:, :],
                in_=bass.AP(tensor=bkv[0:1].tensor, offset=0,
                            ap=[[1, 128], [128, 2]]))

            # ---- weights: single bulk DMA each, already bf16 ----
            # (wq/wkv issued here; wo deferred until after the first input
            # loads so GEMM1(0) isn't stuck behind 2MB it doesn't need)
            wq_t = wpool.tile([128, NDC * CQ], BF16, tag="wq")
            wkv_t = wpool.tile([128, NDC * 2 * CK], BF16, tag="wkv")
            wo_t = wpool.tile([128, 4 * DIM], BF16, tag="wo")

            def load_wq_wkv():
                # scalar queue: runs in parallel with the xq load on the
                # gpsimd queue, so GEMM1(0) starts earlier
                nc.scalar.dma_start(out=wq_t[:, :], in_=wq[:, :])
                nc.scalar.dma_start(out=wkv_t[:, :], in_=wkv[:, :])

            def load_wo():
                nc.sync.dma_start(out=wo_t[:, :], in_=wo[:, :])

            def wq_ap(dc, cc):
                return wq_t[:, dc * CQ + cc * 128:dc * CQ + (cc + 1) * 128]

            def wk_ap(dc):
                return wkv_t[:, dc * 2 * CK:dc * 2 * CK + CK]

            def wv_ap(dc):
                return wkv_t[:, dc * 2 * CK + CK:(dc + 1) * 2 * CK]

            def wo_ap(cc, ec):
                return wo_t[:, cc * DIM + ec * 512:cc * DIM + (ec + 1) * 512]

            # ---- persistent activations ----
            # layout: qxT_ss[ss][cc] [128, 512]: partitions 0:64 = head cc
            # (kv group 0), 64:128 = head cc+4 (kv group 1). Host permutes
            # Wq cols / Wo rows to match.
            vx1 = [acts.tile([128, 130], BF16, tag=f"vp{jb}", name=f"vp{jb}")
                   for jb in range(S // 128)]
            qxT = [[None] * 4 for _ in range(NSS)]
            kxT = [None] * NSS
            attnT = [[None] * 4 for _ in range(NSS)]

            # input stream tiles (one ss in flight per tensor)
            def load_inputs(ss):
                # 3 bulk DMAs; single-buffered tiles: by the time load(ss)
                # is issued, GEMM1(ss-1) has consumed the previous content.
                W = NDC * 512
                xq = xs.tile([128, W], BF16, tag="xq", bufs=1,
                             name=f"xq{ss}")
                nc.gpsimd.dma_start(out=xq[:, :],
                                    in_=qT[:, ss * W:(ss + 1) * W])
                xk = xs.tile([128, W], BF16, tag="xk", bufs=1,
                             name=f"xk{ss}")
                nc.gpsimd.dma_start(out=xk[:, :],
                                    in_=kT[:, ss * W:(ss + 1) * W])
                xv = xs.tile([128, W], BF16, tag="xv", bufs=1,
                             name=f"xv{ss}")
                nc.gpsimd.dma_start(out=xv[:, :],
                                    in_=vT[:, ss * W:(ss + 1) * W])
                return xq, xk, xv

            def gemm1_closures(ss, xq, xk, xv):
                """Returns a list of closures, each issuing one projection
                chain for chunk ss."""
                s0 = ss * 512
                cls = []

                def q_chain(cc):
                    def run():
                        ps = accp.tile([128, 512], F32, tag="acc",
                                       name=f"g1q{ss}{cc}")
                        for dc in range(NDC):
                            nc.tensor.matmul(
                                ps[:, :], wq_ap(dc, cc),
                                xq[:, dc * 512:(dc + 1) * 512],
                                start=(dc == 0), stop=(dc == NDC - 1))
                        if mode == "causal":
                            t = qxp.tile([128, 512], BF16, tag=f"qx{cc}",
                                         name=f"qx{ss}{cc}")
                        else:
                            # flat schedule: all chunks' qxT live at once
                            t = qxp.tile([128, 512], BF16,
                                         tag=f"qx{ss}{cc}", bufs=1,
                                         name=f"qx{ss}{cc}")
                        nc.vector.tensor_scalar_add(t[:, :], ps[:, :],
                                                    bq_t[:, cc:cc + 1])
                        qxT[ss][cc] = t
                    return run

                for cc in range(4):
                    cls.append(q_chain(cc))

                def k_chain():
                    ps = accp.tile([128, 512], F32, tag="acc",
                                   name=f"g1k{ss}")
                    for dc in range(NDC):
                        nc.tensor.matmul(ps[:, :], wk_ap(dc),
                                         xk[:, dc * 512:(dc + 1) * 512],
                                         start=(dc == 0),
                                         stop=(dc == NDC - 1))
                    # kxT persists across the whole kernel (attention at
                    # chunk ss reads all chunks <= ss) -> per-ss tag.
                    t = kxp.tile([128, 512], BF16, tag=f"kx{ss}", bufs=1,
                                 name=f"kx{ss}")
                    nc.vector.tensor_scalar_add(t[:, :], ps[:, :],
                                                bkv_t[:, 0:1])
                    kxT[ss] = t
                cls.append(k_chain)

                def v_chain():
                    ps = accp.tile([128, 512], F32, tag="acc",
                                   name=f"g1v{ss}")
                    for dc in range(NDC):
                        nc.tensor.matmul(ps[:, :], wv_ap(dc),
                                         xv[:, dc * 512:(dc + 1) * 512],
                                         start=(dc == 0),
                                         stop=(dc == NDC - 1))
                    vxs = vxp.tile([128, 512], BF16, tag="vxT",
                                   name=f"vxT{ss}")
                    nc.vector.tensor_scalar_add(vxs[:, :], ps[:, :],
                                                bkv_t[:, 1:2])
                    # transpose to natural [j, c] and pack into vx1 with
                    # ones columns (flash denominator trick)
                    vtp = accp.tile([128, 512], BF16, tag="acc",
                                    name=f"vtp{ss}")
                    for sc in range(4):
                        nc.tensor.transpose(
                            vtp[:, sc * 128:(sc + 1) * 128],
                            vxs[:, sc * 128:(sc + 1) * 128], id_t[:, :])
                    for sc in range(4):
                        vx = vx1[ss * 4 + sc]
                        nc.vector.tensor_copy(
                            vx[:, 0:64], vtp[:, sc * 128:sc * 128 + 64])
                        nc.vector.tensor_copy(
                            vx[:, 65:129],
                            vtp[:, sc * 128 + 64:(sc + 1) * 128])
                        nc.vector.memset(vx[:, 64:65], 1.0)
                        nc.vector.memset(vx[:, 129:130], 1.0)
                cls.append(v_chain)
                return cls

            def gemm4_closures(ss):
                """Output projection for chunk ss (reads attnT[ss])."""
                s0 = ss * 512
                cls = []

                def one(sc):
                    def run():
                        i0 = s0 + sc * 128
                        ob = obp.tile([128, DIM], F16, tag="ob",
                                      name=f"ob{ss}{sc}")
                        for ec in range(4):
                            g4 = accp.tile([128, 512], F32, tag="acc",
                                           name=f"g4{ss}{sc}{ec}")
                            for cc2 in range(4):
                                nc.tensor.matmul(
                                    g4[:, :],
                                    attnT[ss][cc2][:, sc * 128:(sc + 1) * 128],
                                    wo_ap(cc2, ec),
                                    start=(cc2 == 0), stop=(cc2 == 3))
                            nc.vector.tensor_copy(
                                ob[:, ec * 512:(ec + 1) * 512], g4[:, :])
                        nc.sync.dma_start(out=out[i0:i0 + 128, :],
                                          in_=ob[:, :])
                    return run

                for sc in range(4):
                    cls.append(one(sc))
                return cls

            def pair_loop(ss, bg):
                """Attention for i-chunk ss, all 4 head pairs; injects
                background closures from bg between j-steps, spread evenly
                over the chunk's injection slots."""
                s0 = ss * 512
                njb = 4 * (ss + 1) if mode == "causal" else S // 128
                nslots = 4 * njb
                stride = max(1, nslots // max(1, len(bg)))
                slot = [0]

                def maybe_inject():
                    slot[0] += 1
                    if bg and slot[0] % stride == 0:
                        bg.pop(0)()
                for cc in range(4):
                    atl = atp.tile([65, 512], F32, tag="atl",
                                   name=f"atl{ss}{cc}")
                    ath = atp.tile([65, 512], F32, tag="ath",
                                   name=f"ath{ss}{cc}")
                    for jb in range(njb):
                        j0 = jb * 128
                        off = max(0, j0 - s0) if mode == "causal" else 0
                        N = 512 - off
                        kx = kxT[jb // 4]
                        kl = kx[0:64, (jb % 4) * 128:(jb % 4 + 1) * 128]
                        kh = kx[64:128, (jb % 4) * 128:(jb % 4 + 1) * 128]
                        sct = scp.tile([128, 1024], F32, tag="sc",
                                       name=f"sc{ss}{cc}{jb}")
                        nc.tensor.matmul(
                            sct[:, 0:N], kl,
                            qxT[ss][cc][0:64, off:512],
                            start=True, stop=True)
                        nc.tensor.matmul(
                            sct[:, 512:512 + N], kh,
                            qxT[ss][cc][64:128, off:512],
                            start=True, stop=True)
                        if mode == "dense":
                            mb = mbp.tile([128, 512], F32, tag="mb",
                                          name=f"mb{ss}{cc}{jb}")
                            nc.gpsimd.dma_start(
                                out=mb[:, :N],
                                in_=mbias[j0:j0 + 128, s0 + off:s0 + 512])
                            nc.vector.tensor_tensor(
                                sct[:, 0:N], sct[:, 0:N], mb[:, :N],
                                ALU.add)
                            nc.vector.tensor_tensor(
                                sct[:, 512:512 + N], sct[:, 512:512 + N],
                                mb[:, :N], ALU.add)
                        ex = expp.tile([128, 1024], BF16, tag="ex",
                                       name=f"ex{ss}{cc}{jb}")
                        if off == 0:
                            nc.scalar.activation(ex[:, :], sct[:, :],
                                                 AF.Exp, scale=0.125)
                        else:
                            nc.scalar.activation(ex[:, 0:N], sct[:, 0:N],
                                                 AF.Exp, scale=0.125)
                            nc.scalar.activation(ex[:, 512:512 + N],
                                                 sct[:, 512:512 + N],
                                                 AF.Exp, scale=0.125)
                        if mode == "causal" and j0 >= s0:
                            nc.vector.tensor_tensor(
                                ex[:, 0:128], ex[:, 0:128], tri_t[:, :],
                                ALU.mult)
                            nc.vector.tensor_tensor(
                                ex[:, 512:640], ex[:, 512:640], tri_t[:, :],
                                ALU.mult)
                        nc.tensor.matmul(
                            atl[:, off:512], vx1[jb][:, 0:65], ex[:, 0:N],
                            start=(jb == 0), stop=(jb == njb - 1))
                        nc.tensor.matmul(
                            ath[:, off:512], vx1[jb][:, 65:130],
                            ex[:, 512:512 + N],
                            start=(jb == 0), stop=(jb == njb - 1))
                        maybe_inject()
                    # ---- normalize both heads of the pair ----
                    # reciprocal of denominator row, partition-broadcast on
                    # GPSIMD (SBUF->SBUF, keeps PE out of the path), then
                    # one DVE mult per head (PSUM x SBUF -> SBUF).
                    att = anp.tile([128, 512], BF16, tag=f"an{cc}",
                                   name=f"an{ss}{cc}")
                    for hi, at in ((0, atl), (1, ath)):
                        # approx-recip mis-reads PSUM at partition 64;
                        # stage the denominator row through SBUF first.
                        den = nmp.tile([1, 512], F32, tag="den",
                                       name=f"den{ss}{cc}{hi}")
                        nc.vector.tensor_copy(den[0:1, :], at[64:65, :])
                        nmf = nmp.tile([1, 512], F32, tag="nmf",
                                       name=f"nmf{ss}{cc}{hi}")
                        nc.vector.reciprocal_approx_fast(nmf[0:1, :],
                                                         den[0:1, :])
                        nmb = nmp.tile([64, 512], F32, tag="nmb",
                                       name=f"nmb{ss}{cc}{hi}")
                        nc.gpsimd.partition_broadcast(nmb[:, :],
                                                      nmf[0:1, :])
                        nc.vector.tensor_tensor(
                            att[hi * 64:(hi + 1) * 64, :], at[0:64, :],
                            nmb[:, :], ALU.mult)
                    attnT[ss][cc] = att

            # ================= main schedule =================
            if mode == "causal":
                # software-pipelined: GEMM1(ss+1) and (two-chunk-deferred)
                # GEMM4 chains are injected into chunk ss's attention loop
                # to fill PE while ACT computes exp. Later chunks have more
                # ACT work (bigger njb), so GEMM4 lands there.
                xin = load_inputs(0)
                load_wq_wkv()
                for c in gemm1_closures(0, *xin):
                    c()
                load_wo()
                xin = load_inputs(1)
                # per-chunk background work: ss0: G1(1); ss1: G1(2);
                # ss2: G1(3)+G4(0); ss3: G4(1)+G4(2); epilogue: G4(3)
                for ss in range(NSS):
                    bg = []
                    if ss + 1 < NSS:
                        bg.extend(gemm1_closures(ss + 1, *xin))
                    if ss >= 2:
                        bg.extend(gemm4_closures(ss - 2))
                    if ss == NSS - 1:
                        bg.extend(gemm4_closures(ss - 1))
                    pair_loop(ss, bg)
                    for c in bg:
                        c()
                    if ss + 2 < NSS:
                        xin = load_inputs(ss + 2)
                for c in gemm4_closures(NSS - 1):
                    c()
            else:
                # non-causal: attention at any chunk reads ALL k/v chunks,
                # so all projections must complete first (flat schedule).
                load_wq_wkv()
                load_wo()
                for ss in range(NSS):
                    xin = load_inputs(ss)
                    for c in gemm1_closures(ss, *xin):
                        c()
                for ss in range(NSS):
                    bg = gemm4_closures(ss - 1) if ss > 0 else []
                    pair_loop(ss, bg)
                    for c in bg:
                        c()
                for c in gemm4_closures(NSS - 1):
                    c()

    nc.finalize()
    return nc


_CACHE = {}


def _get_nc(mode):
    if mode not in _CACHE:
        _CACHE[mode] = build_nc(mode)
    return _CACHE[mode]


def _to_bf16(x):
    """Fast float32 -> bfloat16 with round-to-nearest-even."""
    x = np.ascontiguousarray(x, np.float32)
    u = x.view(np.uint32)
    r = ((u + 0x7FFF + ((u >> 16) & 1)) >> 16).astype(np.uint16)
    return r.view(ml_dtypes.bfloat16)


def _swz_x(xT_bf16):
    """[DIM, S] transposed tensor -> SBUF-layout [128, NSS*NDC*512]:
    free index = ss*(NDC*512) + dc*512 + s."""
    a = xT_bf16.reshape(NDC, 128, NSS, 512).transpose(1, 2, 0, 3)
    return np.ascontiguousarray(a.reshape(128, NSS * NDC * 512))


def _swz_w(w_bf16, ncols):
    """[nchunks*128, ncols] weight -> SBUF layout [128, nchunks*ncols]."""
    n = w_bf16.shape[0] // 128
    a = w_bf16.reshape(n, 128, ncols).transpose(1, 0, 2)
    return np.ascontiguousarray(a.reshape(128, n * ncols))


def kernel(q, k, v, mask, Wq, bq, Wk, bk, Wv, bv, Wo, bo):
    q = np.asarray(q, np.float32)
    k = np.asarray(k, np.float32)
    v = np.asarray(v, np.float32)
    mask = np.asarray(mask)
    Wq = np.asarray(Wq, np.float32)
    Wk = np.asarray(Wk, np.float32)
    Wv = np.asarray(Wv, np.float32)
    Wo = np.asarray(Wo, np.float32)
    bq = np.asarray(bq, np.float32)
    bk = np.asarray(bk, np.float32)
    bv = np.asarray(bv, np.float32)
    bo = np.asarray(bo, np.float32)

    m = mask.astype(np.float64)
    if np.array_equal(m, np.tril(np.ones((S, S)))):
        mode = "causal"
    elif np.all(m == 1):
        mode = "none"
    else:
        mode = "dense"

    nc = _get_nc(mode)
    tri_np = np.triu(np.ones((128, 128))).astype(ml_dtypes.bfloat16)
    id_np = np.eye(128).astype(ml_dtypes.bfloat16)

    # transposed, bf16, SBUF-swizzled q/k/v, shared across the 4 cores
    # per batch
    qT = [_swz_x(_to_bf16(q[b].T)) for b in range(B)]
    kT = [_swz_x(_to_bf16(k[b].T)) for b in range(B)]
    vT = [_swz_x(_to_bf16(v[b].T)) for b in range(B)]

    # On-chip layout places local q head h in tile h%4 at partition
    # (h//4)*64 so q/k partition bases match in the scores matmul. Permute
    # Wq columns / Wo rows / bq accordingly: tile cc holds heads (cc, cc+4).
    head_perm = [h for cc in range(4) for h in (cc, cc + 4)]
    col_perm = np.concatenate(
        [np.arange(h * HD, (h + 1) * HD) for h in head_perm])

    in_maps = []
    for core in range(NCORES):
        b, kb = core // KVSH, core % KVSH
        wq_sh = Wq[:, kb * CQ:(kb + 1) * CQ][:, col_perm]
        wo_sh = Wo[kb * CQ:(kb + 1) * CQ, :][col_perm, :]
        bq_sh = bq[kb * CQ:(kb + 1) * CQ][col_perm]
        wkv_sh = np.concatenate(
            [Wk[:, kb * CK:(kb + 1) * CK], Wv[:, kb * CK:(kb + 1) * CK]],
            axis=1)
        bkv_sh = np.concatenate(
            [bk[kb * CK:(kb + 1) * CK], bv[kb * CK:(kb + 1) * CK]])
        im = {
            "qT": qT[b],
            "kT": kT[b],
            "vT": vT[b],
            "wq": _swz_w(_to_bf16(wq_sh), CQ),
            "wkv": _swz_w(_to_bf16(wkv_sh), 2 * CK),
            "wo": _swz_w(_to_bf16(wo_sh), DIM),
            "bq": np.ascontiguousarray(bq_sh),
            "bkv": bkv_sh,
            "tri": tri_np,
            "ident": id_np,
        }
        if mode == "dense":
            with np.errstate(divide="ignore"):
                bias = -(1.0 / mask.astype(np.float32) + 1.0)
            im["mbias"] = np.ascontiguousarray(bias.T * 8.0)
        in_maps.append(im)

    res = run_bass_kernel_spmd(nc, in_maps, core_ids=list(range(NCORES)))
    outs = [r["out"] for r in res.results]
    full = np.empty((B, S, DIM), np.float32)
    for b in range(B):
        acc = outs[b * KVSH].astype(np.float32)
        for kb in range(1, KVSH):
            acc = acc + outs[b * KVSH + kb]
        full[b] = acc + bo[None, :]
    return full
